# revision 4
# baseline (speedup 1.0000x reference)
"""Trainium2 Bass kernel for a 2-layer GCN encoder (40000 nodes, 640000 edges,
256 features, 64-graph mean pooling), SPMD across 8 NeuronCores.

Strategy
--------
Math: per layer  z = dinv * Agg(m') ,  m' = dinv * (input @ W)  (self-loops in
the edge list; the bias cancels inside training-mode BatchNorm), then
h = relu(bn(z)).  Output = per-graph mean pool of layer-2 h.

Sharding: nodes are sharded contiguously across the 8 cores (5000 each).  On
each core its 5000 destination nodes are permuted into 40 blocks of 125; the
blocks are split into two GROUPS (0-19, 20-39).  The per-layer node-feature
table is AllGathered in TWO pieces (one per group), so the group-0 collective
fires as soon as the first half of the previous layer's pass-2 finishes, and
aggregation over group-0 sources overlaps the group-1 collective.

Aggregation runs as two sweeps over all 40 dst blocks (one per source part);
each (block, part) contributes NCH padded 128-edge chunks, accumulated in
PSUM by TensorEngine matmuls
   psum[dst 128, feat 256] += onehot[edge 128, dst 128]^T @ gathered[edge 128, feat 256]
with the fp8 one-hot streamed from DRAM and source rows fetched by
`dma_gather` (1024 rows / instruction, 4 SWDGE queues) from the part's
AllGathered fp16 table.  Sweep results are combined in SBUF (fp16) with the
dinv_dst scale applied on the Scalar engine (activation Copy), keeping the
per-block tensor queue free of cross-engine stalls.  BatchNorm statistics are
computed in a separate end-of-layer sweep (ones-mask matmuls), AllReduced,
and applied in pass 2, whose group-0 half feeds the next layer's group-0
AllGather immediately.  Greedy two-iteration balancing assigns nodes to
blocks so per-(block, part) edge counts stay under NCH*128 on every core.
"""

import numpy as np
import ml_dtypes

import concourse.bacc as bacc
import concourse.bass as bass
import concourse.mybir as mybir
import concourse.tile as tile
from concourse import bass_utils

P = 128
F = 256          # feature width (both layers)
NCORES = 8
BN_EPS = 1e-5
NI = 1024        # rows per dma_gather
CPG = NI // P    # chunks per gather = 8

BF16 = mybir.dt.bfloat16
F16 = mybir.dt.float16
F32 = mybir.dt.float32
I16 = mybir.dt.int16
I32 = mybir.dt.int32


class Cfg:
    def __init__(self, N, G, NBLK, NPB):
        assert N == NCORES * NBLK * NPB and NPB <= P
        assert NBLK % 2 == 0
        self.N, self.G, self.NBLK, self.NPB = N, G, NBLK, NPB
        self.NPC = NBLK * NPB          # nodes per core
        self.GBLK = NBLK // 2          # blocks per group
        self.PHALF = N // 2            # rows per part table


# ----------------------------------------------------------------------------
# host-side preprocessing
# ----------------------------------------------------------------------------

def _preprocess(x, ei, batch, W1, g1, be1, W2, g2, be2, cfg):
    N, G, NBLK, NPB, NPC = cfg.N, cfg.G, cfg.NBLK, cfg.NPB, cfg.NPC
    GBLK, PHALF = cfg.GBLK, cfg.PHALF
    PNPC = NPC // 2

    loops = np.arange(N, dtype=np.int64)
    src = np.concatenate([np.asarray(ei[0], dtype=np.int64), loops])
    dst = np.concatenate([np.asarray(ei[1], dtype=np.int64), loops])

    deg = np.bincount(dst, minlength=N).astype(np.float64)
    dinv = (1.0 / np.sqrt(deg)).astype(np.float32)

    node_core = (np.arange(N) // NPC).astype(np.int32)

    # phase 1: pin each node's part (source-side group) up front — per core,
    # alternate by descending degree so both parts carry similar total
    # out-degree; parts then stay FIXED, so dst-side per-part in-degrees are
    # known exactly before block assignment.
    degs = np.bincount(src, minlength=N)          # out-degree (incl. loop)
    part = np.empty(N, np.int64)
    for c in range(NCORES):
        nodes = np.arange(c * NPC, (c + 1) * NPC)
        order = nodes[np.argsort(-degs[nodes], kind="stable")]
        part[order[0::2]] = 0
        part[order[1::2]] = 1

    m0 = part[src] == 0
    deg0 = np.bincount(dst[m0], minlength=N)
    deg1 = np.bincount(dst[~m0], minlength=N)

    # phase 2: per core and per group, greedily assign that group's 2500
    # nodes to its 20 blocks, balancing both per-part in-degree loads.
    blk = np.empty(N, np.int32)
    slot = np.empty(N, np.int32)
    for c in range(NCORES):
        for p in (0, 1):
            nodes = np.arange(c * NPC, (c + 1) * NPC)
            nodes = nodes[part[nodes] == p]
            order = nodes[np.argsort(-(deg0[nodes] + deg1[nodes]),
                                     kind="stable")]
            loadA = np.zeros(GBLK, np.int64)
            loadB = np.zeros(GBLK, np.int64)
            cnt_b = np.zeros(GBLK, np.int64)
            for n in order:
                score = np.maximum(loadA + deg0[n], loadB + deg1[n]).astype(
                    np.float64)
                score[cnt_b >= NPB] = np.inf
                b = int(np.argmin(score))
                blk[n] = p * GBLK + b
                slot[n] = cnt_b[b]
                cnt_b[b] += 1
                loadA[b] += deg0[n]
                loadB[b] += deg1[n]

    # row of each node inside its part's AllGather table
    agrow_p = node_core * PNPC + (blk - part * GBLK) * NPB + slot

    # group edges by (dst core, dst block, src part)
    ecore = (dst // NPC).astype(np.int32)
    eblk = blk[dst]
    edstl = slot[dst]
    epart = part[src].astype(np.int32)
    key = (ecore * NBLK + eblk) * 2 + epart
    order = np.lexsort((src, key))
    s_key = key[order]
    s_src = src[order]
    s_dstl = edstl[order].astype(np.float32)

    counts = np.bincount(key, minlength=NCORES * NBLK * 2)
    NCH = int(np.ceil(counts.max() / P))          # chunks per (block, part)
    SC = NBLK * NCH                               # stream chunks per part
    SCP = ((SC + CPG - 1) // CPG) * CPG           # padded to gather multiple
    NG = SCP // CPG                               # gathers per part-stream

    starts = np.concatenate([[0], np.cumsum(counts)])[:-1]
    rank = np.arange(len(s_key)) - starts[s_key]
    chunkrel = rank // P
    spart = s_key % 2
    score_blk = (s_key // 2) % NBLK               # dst block
    score_core = s_key // (2 * NBLK)              # dst core
    scol = score_blk * NCH + chunkrel             # stream chunk column
    flat = scol * P + (rank % P)                  # position within stream

    hrow = agrow_p[s_src].astype(np.int16)
    valid_f = np.ones(len(s_src), np.float32)

    in_maps = []
    xp = np.asarray(x, dtype=np.float32) * dinv[:, None]
    W1b = np.asarray(W1, dtype=np.float32).reshape(2, P, F).astype(np.float16)
    W2b = np.asarray(W2, dtype=np.float32).reshape(2, P, F).astype(np.float16)
    gb1 = np.concatenate([np.asarray(g1, np.float32),
                          np.asarray(be1, np.float32)])[None, :]
    gb2 = np.concatenate([np.asarray(g2, np.float32),
                          np.asarray(be2, np.float32)])[None, :]
    batch = np.asarray(batch, dtype=np.int64)

    for c in range(NCORES):
        m = {}
        for p in (0, 1):
            sel = (score_core == c) & (spart == p)
            vidx = np.zeros(SCP * P, np.int16)
            vdstl = np.zeros(SCP * P, np.float32)
            vvalid = np.zeros(SCP * P, np.float32)
            f = flat[sel]
            vidx[f] = hrow[sel]
            vdstl[f] = s_dstl[sel]
            vvalid[f] = valid_f[sel]
            # wrap idxs: idx i -> [i%16, i//16], replicated to 128 partitions
            w16 = vidx.reshape(-1, 16).T               # [16, SCP*8]
            m[f"idx{p}"] = np.ascontiguousarray(np.tile(w16, (8, 1)))
            # fp8 one-hot blob: Bb[q, scol*128 + d] = (dstl==d) & valid
            dstl2 = vdstl.reshape(SCP, P).T            # [128, SCP]
            valid = (vvalid.reshape(SCP, P).T != 0.0)
            oneh = (dstl2[:, :, None] ==
                    np.arange(P, dtype=np.float32)[None, None, :]) & valid[:, :, None]
            m[f"bb{p}"] = np.ascontiguousarray(
                oneh.reshape(P, SCP * P)).astype(ml_dtypes.float8_e4m3)

        nodes = np.arange(c * NPC, (c + 1) * NPC)
        col = blk[nodes] * P + slot[nodes]
        ddst = np.zeros((NBLK * P,), np.float32)
        ddst[col] = dinv[nodes]
        m["ddst"] = np.ascontiguousarray(ddst.reshape(NBLK, P).T)   # [128, NBLK]
        bt = np.full((NBLK * P,), 1000.0, np.float32)
        bt[col] = batch[nodes].astype(np.float32)
        m["bt"] = np.ascontiguousarray(bt.reshape(NBLK, P).T)       # [128, NBLK]

        xa = np.zeros((NBLK * P, F), np.float32)
        xa[col] = xp[nodes]
        m["xT"] = np.ascontiguousarray(
            xa.T.reshape(2, P, NBLK * P)).astype(np.float16)

        m["w1"] = W1b
        m["w2"] = W2b
        m["gb1"] = gb1
        m["gb2"] = gb2
        in_maps.append(m)

    cnt = np.bincount(batch, minlength=G).astype(np.float32)
    return in_maps, cnt, NCH, SCP, NG


# ----------------------------------------------------------------------------
# device program
# ----------------------------------------------------------------------------

def _build(cfg, NCH, SCP, NG, reps=1):
    N, G, NBLK, NPB, NPC = cfg.N, cfg.G, cfg.NBLK, cfg.NPB, cfg.NPC
    GBLK, PHALF = cfg.GBLK, cfg.PHALF
    rg = [list(range(NCORES))]

    nc = bacc.Bacc("TRN2", target_bir_lowering=False, debug=False,
                   num_devices=NCORES, num_swdge_queues=4)

    F8 = mybir.dt.float8e4
    din = {}
    for p in (0, 1):
        din[f"idx{p}"] = nc.dram_tensor(f"idx{p}", [P, SCP * 8], I16,
                                        kind="ExternalInput")
        din[f"bb{p}"] = nc.dram_tensor(f"bb{p}", [P, SCP * P], F8,
                                       kind="ExternalInput")
    din["ddst"] = nc.dram_tensor("ddst", [P, NBLK], F32, kind="ExternalInput")
    din["bt"] = nc.dram_tensor("bt", [P, NBLK], F32, kind="ExternalInput")
    din["xT"] = nc.dram_tensor("xT", [2, P, NBLK * P], F16, kind="ExternalInput")
    din["w1"] = nc.dram_tensor("w1", [2, P, F], F16, kind="ExternalInput")
    din["w2"] = nc.dram_tensor("w2", [2, P, F], F16, kind="ExternalInput")
    din["gb1"] = nc.dram_tensor("gb1", [1, 2 * F], F32, kind="ExternalInput")
    din["gb2"] = nc.dram_tensor("gb2", [1, 2 * F], F32, kind="ExternalInput")

    pool_out = nc.dram_tensor("pool_out", [G, F], F32, kind="ExternalOutput")

    ag_in = [[nc.dram_tensor(f"ag_in{l}_{p}", [NPC // 2, F], F16,
                             kind="Internal") for p in (0, 1)] for l in (0, 1)]
    ag_out = [[nc.dram_tensor(f"ag_out{l}_{p}", [PHALF, F], F16,
                              kind="Internal", addr_space="Shared")
               for p in (0, 1)] for l in (0, 1)]
    st_in = [nc.dram_tensor(f"st_in{l}", [1, 2 * F], F32, kind="Internal")
             for l in (0, 1)]
    st_out = [nc.dram_tensor(f"st_out{l}", [1, 2 * F], F32, kind="Internal",
                             addr_space="Shared") for l in (0, 1)]

    with tile.TileContext(nc) as tc:
        import contextlib
        with contextlib.ExitStack() as ctx:
            meta = ctx.enter_context(tc.tile_pool(name="meta", bufs=1))
            big = ctx.enter_context(tc.tile_pool(name="big", bufs=1))
            gpools = [ctx.enter_context(tc.tile_pool(name=f"g{p}", bufs=8))
                      for p in (0, 1)]
            bpool = ctx.enter_context(tc.tile_pool(name="bpool", bufs=8))
            wpool = ctx.enter_context(tc.tile_pool(name="wpool", bufs=3))
            spool = ctx.enter_context(tc.tile_pool(name="spool", bufs=2))
            ps_agg = ctx.enter_context(
                tc.tile_pool(name="ps_agg", bufs=3, space="PSUM"))
            ps_st = ctx.enter_context(
                tc.tile_pool(name="ps_st", bufs=1, space="PSUM"))
            ps_misc = ctx.enter_context(
                tc.tile_pool(name="ps_misc", bufs=2, space="PSUM"))
            ps_pool = ctx.enter_context(
                tc.tile_pool(name="ps_pool", bufs=1, space="PSUM"))

            # --- resident data
            hT1 = [big.tile([P, NBLK * P], F16, tag=f"hT1_{kc}", name=f"hT1_{kc}")
                   for kc in (0, 1)]
            for kc in (0, 1):
                nc.sync.dma_start(out=hT1[kc][:], in_=din["xT"][kc, :, :])
            w_t = []
            for l, name in ((0, "w1"), (1, "w2")):
                tiles = []
                for kc in (0, 1):
                    wt = meta.tile([P, F], F16, tag=f"{name}_{kc}", name=f"{name}_{kc}")
                    nc.sync.dma_start(out=wt[:], in_=din[name][kc, :, :])
                    tiles.append(wt)
                w_t.append(tiles)
            idx_t = []
            for p in (0, 1):
                it = meta.tile([P, SCP * 8], I16, tag=f"idx{p}", name=f"idx{p}")
                nc.sync.dma_start(out=it[:], in_=din[f"idx{p}"][:, :])
                idx_t.append(it)
            ddst_t = meta.tile([P, NBLK], F32, tag="ddst", name="ddst")
            nc.sync.dma_start(out=ddst_t[:], in_=din["ddst"][:, :])
            bt_t = meta.tile([P, NBLK], F32, tag="bt", name="bt")
            nc.sync.dma_start(out=bt_t[:], in_=din["bt"][:, :])

            gb_t = []
            for l, name in ((0, "gb1"), (1, "gb2")):
                gt = meta.tile([1, 2 * F], F32, tag=name, name=name)
                nc.sync.dma_start(out=gt[:], in_=din[name][:, :])
                gb_t.append(gt)

            iota_i = meta.tile([P, P], I32, tag="iota_i", name="iota_i")
            nc.gpsimd.iota(iota_i[:], [[1, P]], channel_multiplier=0)
            iota_f = meta.tile([P, P], F32, tag="iota_f", name="iota_f")
            nc.vector.tensor_copy(out=iota_f[:], in_=iota_i[:])

            from concourse.masks import make_identity
            ident = meta.tile([P, P], F16, tag="ident", name="ident")
            make_identity(nc, ident[:])

            vmask = meta.tile([P, 1], F16, tag="vmask", name="vmask")
            nc.vector.memset(vmask[:], 0.0)
            nc.vector.memset(vmask[0:NPB, :], 1.0)

            eps_t = meta.tile([1, 1], F32, tag="eps_t", name="eps_t")
            nc.vector.memset(eps_t[:], BN_EPS)

            hT2 = [big.tile([P, NBLK * P], F16, tag=f"hT2_{kc}", name=f"hT2_{kc}")
                   for kc in (0, 1)]
            z_all = big.tile([P, NBLK * F], F16, tag="z_all", name="z_all")

            recip_n = 1.0 / float(N)

            def fire_ag(layer, p):
                if DEBUG_NO_CC:
                    nc.sync.dma_start(
                        out=ag_out[layer][p][0:NPC // 2, :],
                        in_=ag_in[layer][p][:, :])
                else:
                    nc.gpsimd.collective_compute(
                        "AllGather", mybir.AluOpType.bypass, replica_groups=rg,
                        ins=[ag_in[layer][p][:, :]],
                        outs=[ag_out[layer][p][:, :]])

            for rep in range(reps):
                # --- layer-0 table: per-group matmuls + early AllGathers ----
                for p in (0, 1):
                    for b in range(p * GBLK, (p + 1) * GBLK):
                        mp = ps_misc.tile([P, F], F32, tag="misc", name="misc")
                        for kc in (0, 1):
                            nc.tensor.matmul(
                                out=mp[:], lhsT=hT1[kc][:, b * P:(b + 1) * P],
                                rhs=w_t[0][kc][:],
                                start=(kc == 0), stop=(kc == 1))
                        m_sb = wpool.tile([P, F], F16, tag="m_sb", name="m_sb")
                        nc.vector.tensor_copy(out=m_sb[:], in_=mp[:])
                        nc.sync.dma_start(
                            out=ag_in[0][p][(b - p * GBLK) * NPB:
                                            (b - p * GBLK + 1) * NPB, :],
                            in_=m_sb[0:NPB, :])
                    fire_ag(0, p)

                for layer in (0, 1):
                    # --- aggregation: one sweep per source part -------------
                    gtiles = {0: {}, 1: {}}
                    btiles = {0: {}, 1: {}}

                    def ensure_gather(p, gi, layer=layer, gtiles=gtiles):
                        if gi in gtiles[p]:
                            return gtiles[p][gi]
                        gt = gpools[p].tile([P, CPG * F], F16, tag=f"gt{p}",
                                            name=f"gt{p}")
                        nc.gpsimd.dma_gather(
                            out_ap=gt[:].rearrange("p (c d) -> p c d", d=F),
                            in_ap=ag_out[layer][p][:, :],
                            idxs_ap=idx_t[p][:, gi * (NI // 16):(gi + 1) * (NI // 16)],
                            num_idxs=NI, num_idxs_reg=NI, elem_size=F,
                            queue_num=gi % 4)
                        gtiles[p][gi] = gt
                        return gt

                    def ensure_btile(p, gi, btiles=btiles):
                        if gi in btiles[p]:
                            return btiles[p][gi]
                        bt_ = bpool.tile([P, CPG * P], F8, tag=f"bb{p}",
                                         name=f"bb{p}")
                        nc.sync.dma_start(
                            out=bt_[:],
                            in_=din[f"bb{p}"][:, gi * CPG * P:(gi + 1) * CPG * P])
                        btiles[p][gi] = bt_
                        return bt_

                    for p in (0, 1):
                        for b in range(NBLK):
                            agg = ps_agg.tile([P, F], F32, tag="agg", name="agg")
                            for j in range(NCH):
                                scol = b * NCH + j
                                gi, gslot = divmod(scol, CPG)
                                gt = ensure_gather(p, gi)
                                bt_ = ensure_btile(p, gi)
                                nc.tensor.matmul(
                                    out=agg[:],
                                    lhsT=bt_[:, gslot * P:(gslot + 1) * P],
                                    rhs=gt[:, gslot * F:(gslot + 1) * F],
                                    start=(j == 0), stop=(j == NCH - 1))
                            zsl = z_all[:, b * F:(b + 1) * F]
                            if p == 0:
                                nc.scalar.activation(
                                    out=zsl, in_=agg[:],
                                    func=mybir.ActivationFunctionType.Copy,
                                    scale=ddst_t[:, b:b + 1])
                            else:
                                zt = wpool.tile([P, F], F16, tag="zt",
                                                name="zt", bufs=4)
                                nc.scalar.activation(
                                    out=zt[:], in_=agg[:],
                                    func=mybir.ActivationFunctionType.Copy,
                                    scale=ddst_t[:, b:b + 1])
                                nc.vector.tensor_tensor(
                                    out=zsl, in0=zt[:], in1=zsl,
                                    op=mybir.AluOpType.add)

                    # --- BN stats: end-of-layer sweep ------------------------
                    ssum = ps_st.tile([1, F], F32, tag="ssum", name="ssum")
                    ssq = ps_st.tile([1, F], F32, tag="ssq", name="ssq")
                    for b in range(NBLK):
                        zsl = z_all[:, b * F:(b + 1) * F]
                        sq_t = wpool.tile([P, F], F16, tag="sq_t", name="sq_t",
                                          bufs=4)
                        nc.vector.tensor_tensor(out=sq_t[:], in0=zsl, in1=zsl,
                                                op=mybir.AluOpType.mult)
                        nc.tensor.matmul(out=ssum[:], lhsT=vmask[:], rhs=zsl,
                                         start=(b == 0), stop=(b == NBLK - 1))
                        nc.tensor.matmul(out=ssq[:], lhsT=vmask[:], rhs=sq_t[:],
                                         start=(b == 0), stop=(b == NBLK - 1))

                    # --- stats AllReduce + scale/shift ------------------------
                    srow = spool.tile([1, 2 * F], F32, tag="srow", name="srow")
                    nc.vector.tensor_copy(out=srow[:, 0:F], in_=ssum[:])
                    nc.vector.tensor_copy(out=srow[:, F:2 * F], in_=ssq[:])
                    nc.sync.dma_start(out=st_in[layer][:, :], in_=srow[:])
                    if DEBUG_NO_CC:
                        nc.sync.dma_start(out=st_out[layer][:, :],
                                          in_=st_in[layer][:, :])
                    else:
                        nc.gpsimd.collective_compute(
                            "AllReduce", mybir.AluOpType.add, replica_groups=rg,
                            ins=[st_in[layer][:, :]], outs=[st_out[layer][:, :]])
                    srow2 = spool.tile([1, 2 * F], F32, tag="srow2", name="srow2")
                    nc.sync.dma_start(out=srow2[:], in_=st_out[layer][:, :])

                    mu = spool.tile([1, F], F32, tag="mu", name="mu")
                    nc.vector.tensor_scalar(out=mu[:], in0=srow2[:, 0:F],
                                            scalar1=recip_n, scalar2=None,
                                            op0=mybir.AluOpType.mult)
                    ex2 = spool.tile([1, F], F32, tag="ex2", name="ex2")
                    nc.vector.tensor_scalar(out=ex2[:], in0=srow2[:, F:2 * F],
                                            scalar1=recip_n, scalar2=None,
                                            op0=mybir.AluOpType.mult)
                    var = spool.tile([1, F], F32, tag="var", name="var")
                    nc.vector.tensor_tensor(out=var[:], in0=mu[:], in1=mu[:],
                                            op=mybir.AluOpType.mult)
                    nc.vector.tensor_tensor(out=var[:], in0=ex2[:], in1=var[:],
                                            op=mybir.AluOpType.subtract)
                    sd = spool.tile([1, F], F32, tag="sd", name="sd")
                    nc.scalar.activation(out=sd[:], in_=var[:],
                                         func=mybir.ActivationFunctionType.Sqrt,
                                         bias=eps_t[:])
                    rstd = spool.tile([1, F], F32, tag="rstd", name="rstd")
                    nc.vector.reciprocal(rstd[:], sd[:])
                    s_row = spool.tile([1, F], F32, tag="s_row", name="s_row")
                    nc.vector.tensor_tensor(out=s_row[:], in0=rstd[:],
                                            in1=gb_t[layer][:, 0:F],
                                            op=mybir.AluOpType.mult)
                    t_row = spool.tile([1, F], F32, tag="t_row", name="t_row")
                    nc.vector.tensor_tensor(out=t_row[:], in0=mu[:], in1=s_row[:],
                                            op=mybir.AluOpType.mult)
                    nc.vector.tensor_tensor(out=t_row[:], in0=gb_t[layer][:, F:2 * F],
                                            in1=t_row[:],
                                            op=mybir.AluOpType.subtract)
                    S_b = spool.tile([P, F], F32, tag="S_b", name="S_b")
                    nc.gpsimd.partition_broadcast(out_ap=S_b[:], in_ap=s_row[:])
                    T_b = spool.tile([P, F], F32, tag="T_b", name="T_b")
                    nc.gpsimd.partition_broadcast(out_ap=T_b[:], in_ap=t_row[:])

                    # --- pass 2: h = relu(z*S + T), feed next stage ----------
                    if layer == 1:
                        pool_ps = ps_pool.tile([G, F], F32, tag="pool", name="pool")
                    for p in (0, 1):
                        for b in range(p * GBLK, (p + 1) * GBLK):
                            zsl = z_all[:, b * F:(b + 1) * F]
                            u = wpool.tile([P, F], F16, tag="u", name="u", bufs=4)
                            nc.vector.tensor_tensor(out=u[:], in0=zsl, in1=S_b[:],
                                                    op=mybir.AluOpType.mult)
                            u2 = wpool.tile([P, F], F16, tag="u2", name="u2", bufs=4)
                            nc.vector.tensor_tensor(out=u2[:], in0=u[:], in1=T_b[:],
                                                    op=mybir.AluOpType.add)
                            hp = wpool.tile([P, F], F16, tag="hp", name="hp")
                            if layer == 0:
                                nc.scalar.activation(
                                    out=hp[:], in_=u2[:],
                                    func=mybir.ActivationFunctionType.Relu,
                                    scale=ddst_t[:, b:b + 1])
                                for kc in (0, 1):
                                    tp = ps_misc.tile([P, P], F16, tag="misc", name="misc")
                                    nc.tensor.transpose(
                                        out=tp[:], in_=hp[:, kc * P:(kc + 1) * P],
                                        identity=ident[:])
                                    nc.vector.tensor_copy(
                                        out=hT2[kc][:, b * P:(b + 1) * P], in_=tp[:])
                                mp2 = ps_misc.tile([P, F], F32, tag="misc", name="misc")
                                for kc in (0, 1):
                                    nc.tensor.matmul(
                                        out=mp2[:], lhsT=hT2[kc][:, b * P:(b + 1) * P],
                                        rhs=w_t[1][kc][:],
                                        start=(kc == 0), stop=(kc == 1))
                                m_sb2 = wpool.tile([P, F], F16, tag="m_sb", name="m_sb")
                                nc.vector.tensor_copy(out=m_sb2[:], in_=mp2[:])
                                nc.sync.dma_start(
                                    out=ag_in[1][p][(b - p * GBLK) * NPB:
                                                    (b - p * GBLK + 1) * NPB, :],
                                    in_=m_sb2[0:NPB, :])
                            else:
                                nc.scalar.activation(
                                    out=hp[:], in_=u2[:],
                                    func=mybir.ActivationFunctionType.Relu)
                                pone = bpool.tile([P, G], F16, tag="pone", name="pone")
                                nc.vector.tensor_scalar(
                                    out=pone[:], in0=iota_f[:, 0:G],
                                    scalar1=bt_t[:, b:b + 1], scalar2=None,
                                    op0=mybir.AluOpType.is_equal)
                                nc.tensor.matmul(out=pool_ps[:], lhsT=pone[:],
                                                 rhs=hp[:], start=(b == 0),
                                                 stop=(b == NBLK - 1))
                        if layer == 0:
                            fire_ag(1, p)

                pool_sb = spool.tile([G, F], F32, tag="pool_sb", name="pool_sb")
                nc.vector.tensor_copy(out=pool_sb[:], in_=pool_ps[:])
                nc.sync.dma_start(out=pool_out[:, :], in_=pool_sb[:])

    nc.compile()
    return nc


DEBUG_NO_CC = False   # replace collectives with local DMA (timing-only builds)

_CACHE = {}


def _get_program(cfg, NCH, SCP, NG, reps=1):
    key = (cfg.N, cfg.G, cfg.NBLK, cfg.NPB, NCH, SCP, NG, reps)
    if key not in _CACHE:
        _CACHE[key] = _build(cfg, NCH, SCP, NG, reps)
    return _CACHE[key]


def _run(inputs, cfg, trace=False):
    in_maps, cnt, NCH, SCP, NG = _preprocess(
        inputs["x"], inputs["ei"], inputs["batch"],
        inputs["W1"], inputs["g1"], inputs["be1"],
        inputs["W2"], inputs["g2"], inputs["be2"], cfg)
    nc = _get_program(cfg, NCH, SCP, NG)
    res = bass_utils.run_bass_kernel_spmd(
        nc, in_maps, core_ids=list(range(NCORES)), trace=trace)
    partial = np.zeros((cfg.G, F), np.float32)
    for c in range(NCORES):
        partial += np.asarray(res.results[c]["pool_out"], np.float32)
    out = partial / np.maximum(cnt, 1.0)[:, None]
    return out.astype(np.float32), res


def kernel(**inputs):
    cfg = Cfg(N=40000, G=64, NBLK=40, NPB=125)
    out, _ = _run(inputs, cfg)
    return out


# revision 7
# speedup vs baseline: 2.1509x; 2.1509x over previous
"""Trainium2 Bass kernel for a 2-layer GCN encoder (40000 nodes, 640000 edges,
256 features, 64-graph mean pooling), SPMD across 8 NeuronCores.

Strategy
--------
Math: per layer  z = dinv * Agg(m') ,  m' = dinv * (input @ W)  (self-loops in
the edge list; the bias cancels inside training-mode BatchNorm), then
h = relu(bn(z)).  Output = per-graph mean pool of layer-2 h.

Sharding: nodes are sharded contiguously across the 8 cores (5000 each).  On
each core its 5000 destination nodes are permuted into 40 blocks of 125; the
blocks are split into two GROUPS (0-19, 20-39).  The per-layer node-feature
table is AllGathered in TWO pieces (one per group), so the group-0 collective
fires as soon as the first half of the previous layer's pass-2 finishes, and
aggregation over group-0 sources overlaps the group-1 collective.

Aggregation runs as two sweeps over all 40 dst blocks (one per source part);
each (block, part) contributes NCH padded 128-edge chunks, accumulated in
PSUM by TensorEngine matmuls
   psum[dst 128, feat 256] += onehot[edge 128, dst 128]^T @ gathered[edge 128, feat 256]
with the fp8 one-hot streamed from DRAM and source rows fetched by
`dma_gather` (1024 rows / instruction, 4 SWDGE queues) from the part's
AllGathered fp16 table.  Sweep results are combined in SBUF (fp16) with the
dinv_dst scale applied on the Scalar engine (activation Copy), keeping the
per-block tensor queue free of cross-engine stalls.  BatchNorm statistics are
computed in a separate end-of-layer sweep (ones-mask matmuls), AllReduced,
and applied in pass 2, whose group-0 half feeds the next layer's group-0
AllGather immediately.  Greedy two-iteration balancing assigns nodes to
blocks so per-(block, part) edge counts stay under NCH*128 on every core.
"""

import numpy as np
import ml_dtypes

import concourse.bacc as bacc
import concourse.bass as bass
import concourse.mybir as mybir
import concourse.tile as tile
from concourse import bass_utils

P = 128
F = 256          # feature width (both layers)
NCORES = 8
BN_EPS = 1e-5
NI = 1024        # rows per dma_gather
CPG = NI // P    # chunks per gather = 8

BF16 = mybir.dt.bfloat16
F16 = mybir.dt.float16
F32 = mybir.dt.float32
I16 = mybir.dt.int16
I32 = mybir.dt.int32


class Cfg:
    def __init__(self, N, G, NBLK, NPB):
        assert N == NCORES * NBLK * NPB and NPB <= P
        assert NBLK % 2 == 0
        self.N, self.G, self.NBLK, self.NPB = N, G, NBLK, NPB
        self.NPC = NBLK * NPB          # nodes per core
        self.GBLK = NBLK // 2          # blocks per group
        self.PHALF = N // 2            # rows per part table


# ----------------------------------------------------------------------------
# host-side preprocessing
# ----------------------------------------------------------------------------

def _preprocess(x, ei, batch, W1, g1, be1, W2, g2, be2, cfg):
    N, G, NBLK, NPB, NPC = cfg.N, cfg.G, cfg.NBLK, cfg.NPB, cfg.NPC
    GBLK, PHALF = cfg.GBLK, cfg.PHALF
    PNPC = NPC // 2

    loops = np.arange(N, dtype=np.int64)
    src = np.asarray(ei[0], dtype=np.int64)
    dst = np.asarray(ei[1], dtype=np.int64)

    # degree includes the self-loop; the loop's contribution itself is added
    # on-device from the resident transposed tables (two matmuls per block)
    deg = (np.bincount(dst, minlength=N) + 1).astype(np.float64)
    dinv = (1.0 / np.sqrt(deg)).astype(np.float32)

    node_core = (np.arange(N) // NPC).astype(np.int32)

    # phase 1: pin each node's part (source-side group) up front — per core,
    # alternate by descending degree so both parts carry similar total
    # out-degree; parts then stay FIXED, so dst-side per-part in-degrees are
    # known exactly before block assignment.
    degs = np.bincount(src, minlength=N)          # out-degree (incl. loop)
    part = np.empty(N, np.int64)
    for c in range(NCORES):
        nodes = np.arange(c * NPC, (c + 1) * NPC)
        order = nodes[np.argsort(-degs[nodes], kind="stable")]
        part[order[0::2]] = 0
        part[order[1::2]] = 1

    m0 = part[src] == 0
    deg0 = np.bincount(dst[m0], minlength=N)
    deg1 = np.bincount(dst[~m0], minlength=N)

    # phase 2: per core and per group, greedily assign that group's 2500
    # nodes to its 20 blocks, balancing both per-part in-degree loads.
    blk = np.empty(N, np.int32)
    slot = np.empty(N, np.int32)
    for c in range(NCORES):
        for p in (0, 1):
            nodes = np.arange(c * NPC, (c + 1) * NPC)
            nodes = nodes[part[nodes] == p]
            order = nodes[np.argsort(-(deg0[nodes] + deg1[nodes]),
                                     kind="stable")]
            loadA = np.zeros(GBLK, np.int64)
            loadB = np.zeros(GBLK, np.int64)
            cnt_b = np.zeros(GBLK, np.int64)
            for n in order:
                score = np.maximum(loadA + deg0[n], loadB + deg1[n]).astype(
                    np.float64)
                score[cnt_b >= NPB] = np.inf
                b = int(np.argmin(score))
                blk[n] = p * GBLK + b
                slot[n] = cnt_b[b]
                cnt_b[b] += 1
                loadA[b] += deg0[n]
                loadB[b] += deg1[n]

    # local swap repair: push per-(block, part) loads under 8*P if possible
    degP = np.stack([deg0, deg1], axis=1)
    L = np.zeros((NCORES * NBLK, 2), np.int64)
    for p in (0, 1):
        np.add.at(L[:, p], node_core * NBLK + blk, degP[:, p])
    gb = node_core * NBLK + blk
    order_m = np.argsort(gb, kind="stable")
    bounds = np.searchsorted(gb[order_m], np.arange(NCORES * NBLK + 1))
    members = {cb: list(order_m[bounds[cb]:bounds[cb + 1]])
               for cb in range(NCORES * NBLK)}
    target = 8 * P
    for _ in range(3000):
        worst = int(np.argmax(L.max(axis=1)))
        if L[worst].max() <= target:
            break
        p_hot = int(np.argmax(L[worst]))
        c = worst // NBLK
        grp = (worst % NBLK) // GBLK
        best = None
        mem_w = sorted(members[worst], key=lambda n: -degP[n, p_hot])[:12]
        for j in range(GBLK):
            b2 = c * NBLK + grp * GBLK + j
            if b2 == worst:
                continue
            mem2 = sorted(members[b2], key=lambda n: degP[n, p_hot])[:12]
            for n in mem_w:
                for m in mem2:
                    nm = max(L[worst][0] - degP[n, 0] + degP[m, 0],
                             L[worst][1] - degP[n, 1] + degP[m, 1],
                             L[b2][0] + degP[n, 0] - degP[m, 0],
                             L[b2][1] + degP[n, 1] - degP[m, 1])
                    if best is None or nm < best[0]:
                        best = (nm, n, m, b2)
        if best is None or best[0] >= L[worst].max():
            break
        _, n, m, b2 = best
        members[worst].remove(n); members[b2].remove(m)
        members[worst].append(m); members[b2].append(n)
        for p in (0, 1):
            L[worst][p] += degP[m, p] - degP[n, p]
            L[b2][p] += degP[n, p] - degP[m, p]
        blk[n], blk[m] = blk[m], blk[n]
        slot[n], slot[m] = slot[m], slot[n]

    # row of each node inside its part's AllGather table
    agrow_p = node_core * PNPC + (blk - part * GBLK) * NPB + slot

    # group edges by (dst core, dst block, src part)
    ecore = (dst // NPC).astype(np.int32)
    eblk = blk[dst]
    edstl = slot[dst]
    epart = part[src].astype(np.int32)
    key = (ecore * NBLK + eblk) * 2 + epart
    order = np.lexsort((src, key))
    s_key = key[order]
    s_src = src[order]
    s_dstl = edstl[order].astype(np.float32)

    counts = np.bincount(key, minlength=NCORES * NBLK * 2)
    NCH = int(np.ceil(counts.max() / P))          # chunks per (block, part)
    SC = NBLK * NCH                               # stream chunks per part
    SCP = ((SC + CPG - 1) // CPG) * CPG           # padded to gather multiple
    NG = SCP // CPG                               # gathers per part-stream

    starts = np.concatenate([[0], np.cumsum(counts)])[:-1]
    rank = np.arange(len(s_key)) - starts[s_key]
    chunkrel = rank // P
    spart = s_key % 2
    score_blk = (s_key // 2) % NBLK               # dst block
    score_core = s_key // (2 * NBLK)              # dst core
    scol = score_blk * NCH + chunkrel             # stream chunk column
    flat = scol * P + (rank % P)                  # position within stream

    hrow = agrow_p[s_src].astype(np.int16)
    valid_f = np.ones(len(s_src), np.float32)

    in_maps = []
    xp = np.asarray(x, dtype=np.float32) * dinv[:, None]
    W1b = np.asarray(W1, dtype=np.float32).reshape(2, P, F).astype(np.float16)
    W2b = np.asarray(W2, dtype=np.float32).reshape(2, P, F).astype(np.float16)
    gb1 = np.concatenate([np.asarray(g1, np.float32),
                          np.asarray(be1, np.float32)])[None, :]
    gb2 = np.concatenate([np.asarray(g2, np.float32),
                          np.asarray(be2, np.float32)])[None, :]
    batch = np.asarray(batch, dtype=np.int64)

    for c in range(NCORES):
        m = {}
        for p in (0, 1):
            sel = (score_core == c) & (spart == p)
            vidx = np.zeros(SCP * P, np.int16)
            vdstl = np.zeros(SCP * P, np.float32)
            vvalid = np.zeros(SCP * P, np.float32)
            f = flat[sel]
            vidx[f] = hrow[sel]
            vdstl[f] = s_dstl[sel]
            vvalid[f] = valid_f[sel]
            # wrap idxs: idx i -> [i%16, i//16], replicated to 128 partitions
            w16 = vidx.reshape(-1, 16).T               # [16, SCP*8]
            m[f"idx{p}"] = np.ascontiguousarray(np.tile(w16, (8, 1)))
            # fp8 one-hot blob: Bb[q, scol*128 + d] = (dstl==d) & valid
            dstl2 = vdstl.reshape(SCP, P).T            # [128, SCP]
            valid = (vvalid.reshape(SCP, P).T != 0.0)
            oneh = (dstl2[:, :, None] ==
                    np.arange(P, dtype=np.float32)[None, None, :]) & valid[:, :, None]
            m[f"bb{p}"] = np.ascontiguousarray(
                oneh.reshape(P, SCP * P)).astype(ml_dtypes.float8_e4m3)

        nodes = np.arange(c * NPC, (c + 1) * NPC)
        col = blk[nodes] * P + slot[nodes]
        ddst = np.zeros((NBLK * P,), np.float32)
        ddst[col] = dinv[nodes]
        m["ddst"] = np.ascontiguousarray(ddst.reshape(NBLK, P).T)   # [128, NBLK]
        bt = np.full((NBLK * P,), 1000.0, np.float32)
        bt[col] = batch[nodes].astype(np.float32)
        m["bt"] = np.ascontiguousarray(bt.reshape(NBLK, P).T)       # [128, NBLK]

        xa = np.zeros((NBLK * P, F), np.float32)
        xa[col] = xp[nodes]
        m["xT"] = np.ascontiguousarray(
            xa.T.reshape(2, P, NBLK * P)).astype(np.float16)

        m["w1"] = W1b
        m["w2"] = W2b
        m["gb1"] = gb1
        m["gb2"] = gb2
        in_maps.append(m)

    cnt = np.bincount(batch, minlength=G).astype(np.float32)
    return in_maps, cnt, NCH, SCP, NG


# ----------------------------------------------------------------------------
# device program
# ----------------------------------------------------------------------------

def _build(cfg, NCH, SCP, NG, reps=1):
    N, G, NBLK, NPB, NPC = cfg.N, cfg.G, cfg.NBLK, cfg.NPB, cfg.NPC
    GBLK, PHALF = cfg.GBLK, cfg.PHALF
    rg = [list(range(NCORES))]

    nc = bacc.Bacc("TRN2", target_bir_lowering=False, debug=False,
                   num_devices=NCORES, num_swdge_queues=4)

    F8 = mybir.dt.float8e4
    din = {}
    for p in (0, 1):
        din[f"idx{p}"] = nc.dram_tensor(f"idx{p}", [P, SCP * 8], I16,
                                        kind="ExternalInput")
        din[f"bb{p}"] = nc.dram_tensor(f"bb{p}", [P, SCP * P], F8,
                                       kind="ExternalInput")
    din["ddst"] = nc.dram_tensor("ddst", [P, NBLK], F32, kind="ExternalInput")
    din["bt"] = nc.dram_tensor("bt", [P, NBLK], F32, kind="ExternalInput")
    din["xT"] = nc.dram_tensor("xT", [2, P, NBLK * P], F16, kind="ExternalInput")
    din["w1"] = nc.dram_tensor("w1", [2, P, F], F16, kind="ExternalInput")
    din["w2"] = nc.dram_tensor("w2", [2, P, F], F16, kind="ExternalInput")
    din["gb1"] = nc.dram_tensor("gb1", [1, 2 * F], F32, kind="ExternalInput")
    din["gb2"] = nc.dram_tensor("gb2", [1, 2 * F], F32, kind="ExternalInput")

    pool_out = nc.dram_tensor("pool_out", [G, F], F32, kind="ExternalOutput")

    ag_in = [[nc.dram_tensor(f"ag_in{l}_{p}", [NPC // 2, F], F16,
                             kind="Internal") for p in (0, 1)] for l in (0, 1)]
    ag_out = [[nc.dram_tensor(f"ag_out{l}_{p}", [PHALF, F], F16,
                              kind="Internal", addr_space="Shared")
               for p in (0, 1)] for l in (0, 1)]
    st_in = [nc.dram_tensor(f"st_in{l}", [1, 2 * F], F32, kind="Internal")
             for l in (0, 1)]
    st_out = [nc.dram_tensor(f"st_out{l}", [1, 2 * F], F32, kind="Internal",
                             addr_space="Shared") for l in (0, 1)]

    with tile.TileContext(nc) as tc:
        import contextlib
        with contextlib.ExitStack() as ctx:
            meta = ctx.enter_context(tc.tile_pool(name="meta", bufs=1))
            big = ctx.enter_context(tc.tile_pool(name="big", bufs=1))
            gpools = [ctx.enter_context(tc.tile_pool(name=f"g{p}", bufs=9))
                      for p in (0, 1)]
            bpool = ctx.enter_context(tc.tile_pool(name="bpool", bufs=8))
            wpool = ctx.enter_context(tc.tile_pool(name="wpool", bufs=3))
            spool = ctx.enter_context(tc.tile_pool(name="spool", bufs=2))
            ps_agg = ctx.enter_context(
                tc.tile_pool(name="ps_agg", bufs=3, space="PSUM"))
            ps_st = ctx.enter_context(
                tc.tile_pool(name="ps_st", bufs=1, space="PSUM"))
            ps_misc = ctx.enter_context(
                tc.tile_pool(name="ps_misc", bufs=2, space="PSUM"))
            ps_pool = ctx.enter_context(
                tc.tile_pool(name="ps_pool", bufs=1, space="PSUM"))

            # --- resident data
            hT1 = [big.tile([P, NBLK * P], F16, tag=f"hT1_{kc}", name=f"hT1_{kc}")
                   for kc in (0, 1)]
            for kc in (0, 1):
                nc.sync.dma_start(out=hT1[kc][:], in_=din["xT"][kc, :, :])
            w_t = []
            for l, name in ((0, "w1"), (1, "w2")):
                tiles = []
                for kc in (0, 1):
                    wt = meta.tile([P, F], F16, tag=f"{name}_{kc}", name=f"{name}_{kc}")
                    nc.sync.dma_start(out=wt[:], in_=din[name][kc, :, :])
                    tiles.append(wt)
                w_t.append(tiles)
            idx_t = []
            for p in (0, 1):
                it = meta.tile([P, SCP * 8], I16, tag=f"idx{p}", name=f"idx{p}")
                nc.sync.dma_start(out=it[:], in_=din[f"idx{p}"][:, :])
                idx_t.append(it)
            ddst_t = meta.tile([P, NBLK], F32, tag="ddst", name="ddst")
            nc.sync.dma_start(out=ddst_t[:], in_=din["ddst"][:, :])
            bt_t = meta.tile([P, NBLK], F32, tag="bt", name="bt")
            nc.sync.dma_start(out=bt_t[:], in_=din["bt"][:, :])

            gb_t = []
            for l, name in ((0, "gb1"), (1, "gb2")):
                gt = meta.tile([1, 2 * F], F32, tag=name, name=name)
                nc.sync.dma_start(out=gt[:], in_=din[name][:, :])
                gb_t.append(gt)

            iota_i = meta.tile([P, P], I32, tag="iota_i", name="iota_i")
            nc.gpsimd.iota(iota_i[:], [[1, P]], channel_multiplier=0)
            iota_f = meta.tile([P, P], F32, tag="iota_f", name="iota_f")
            nc.vector.tensor_copy(out=iota_f[:], in_=iota_i[:])

            from concourse.masks import make_identity
            ident = meta.tile([P, P], F16, tag="ident", name="ident")
            make_identity(nc, ident[:])

            vmask = meta.tile([P, 1], F16, tag="vmask", name="vmask")
            nc.vector.memset(vmask[:], 0.0)
            nc.vector.memset(vmask[0:NPB, :], 1.0)

            eps_t = meta.tile([1, 1], F32, tag="eps_t", name="eps_t")
            nc.vector.memset(eps_t[:], BN_EPS)

            hT2 = [big.tile([P, NBLK * P], F16, tag=f"hT2_{kc}", name=f"hT2_{kc}")
                   for kc in (0, 1)]
            z_all = big.tile([P, NBLK * F], F16, tag="z_all", name="z_all")

            recip_n = 1.0 / float(N)

            def fire_ag(layer, p):
                if DEBUG_NO_CC:
                    nc.sync.dma_start(
                        out=ag_out[layer][p][0:NPC // 2, :],
                        in_=ag_in[layer][p][:, :])
                else:
                    nc.gpsimd.collective_compute(
                        "AllGather", mybir.AluOpType.bypass, replica_groups=rg,
                        ins=[ag_in[layer][p][:, :]],
                        outs=[ag_out[layer][p][:, :]])

            for rep in range(reps):
                # --- layer-0 table: per-group matmuls + early AllGathers ----
                for p in (0, 1):
                    for b in range(p * GBLK, (p + 1) * GBLK):
                        mp = ps_misc.tile([P, F], F32, tag="misc", name="misc")
                        for kc in (0, 1):
                            nc.tensor.matmul(
                                out=mp[:], lhsT=hT1[kc][:, b * P:(b + 1) * P],
                                rhs=w_t[0][kc][:],
                                start=(kc == 0), stop=(kc == 1))
                        m_sb = wpool.tile([P, F], F16, tag="m_sb", name="m_sb")
                        nc.vector.tensor_copy(out=m_sb[:], in_=mp[:])
                        nc.sync.dma_start(
                            out=ag_in[0][p][(b - p * GBLK) * NPB:
                                            (b - p * GBLK + 1) * NPB, :],
                            in_=m_sb[0:NPB, :])
                    if p == 0:
                        fire_ag(0, 0)

                for layer in (0, 1):
                    # --- aggregation: one sweep per source part -------------
                    gtiles = {0: {}, 1: {}}
                    btiles = {0: {}, 1: {}}

                    def ensure_gather(p, gi, layer=layer, gtiles=gtiles):
                        if gi in gtiles[p]:
                            return gtiles[p][gi]
                        gt = gpools[p].tile([P, CPG * F], F16, tag=f"gt{p}",
                                            name=f"gt{p}")
                        nc.gpsimd.dma_gather(
                            out_ap=gt[:].rearrange("p (c d) -> p c d", d=F),
                            in_ap=ag_out[layer][p][:, :],
                            idxs_ap=idx_t[p][:, gi * (NI // 16):(gi + 1) * (NI // 16)],
                            num_idxs=NI, num_idxs_reg=NI, elem_size=F,
                            queue_num=gi % 4)
                        gtiles[p][gi] = gt
                        return gt

                    def ensure_btile(p, gi, btiles=btiles):
                        if gi in btiles[p]:
                            return btiles[p][gi]
                        bt_ = bpool.tile([P, CPG * P], F8, tag=f"bb{p}",
                                         name=f"bb{p}")
                        nc.sync.dma_start(
                            out=bt_[:],
                            in_=din[f"bb{p}"][:, gi * CPG * P:(gi + 1) * CPG * P])
                        btiles[p][gi] = bt_
                        return bt_

                    hTl = hT1 if layer == 0 else hT2
                    for p in (0, 1):
                        if p == 1:
                            fire_ag(layer, 1)
                        for b in range(NBLK):
                            agg = ps_agg.tile([P, F], F32, tag="agg", name="agg")
                            if p == 0:
                                # self-loop term: this block's own table rows,
                                # recomputed from the resident transposed input
                                for kc in (0, 1):
                                    nc.tensor.matmul(
                                        out=agg[:],
                                        lhsT=hTl[kc][:, b * P:(b + 1) * P],
                                        rhs=w_t[layer][kc][:],
                                        start=(kc == 0), stop=False)
                            for j in range(NCH):
                                scol = b * NCH + j
                                gi, gslot = divmod(scol, CPG)
                                gt = ensure_gather(p, gi)
                                bt_ = ensure_btile(p, gi)
                                nc.tensor.matmul(
                                    out=agg[:],
                                    lhsT=bt_[:, gslot * P:(gslot + 1) * P],
                                    rhs=gt[:, gslot * F:(gslot + 1) * F],
                                    start=(p == 0 and False) or (p == 1 and j == 0),
                                    stop=(j == NCH - 1))
                            zsl = z_all[:, b * F:(b + 1) * F]
                            if p == 0:
                                nc.scalar.activation(
                                    out=zsl, in_=agg[:],
                                    func=mybir.ActivationFunctionType.Copy,
                                    scale=ddst_t[:, b:b + 1])
                            else:
                                zt = wpool.tile([P, F], F16, tag="zt",
                                                name="zt", bufs=4)
                                nc.scalar.activation(
                                    out=zt[:], in_=agg[:],
                                    func=mybir.ActivationFunctionType.Copy,
                                    scale=ddst_t[:, b:b + 1])
                                nc.vector.tensor_tensor(
                                    out=zsl, in0=zt[:], in1=zsl,
                                    op=mybir.AluOpType.add)

                    # --- BN stats: end-of-layer sweep ------------------------
                    ssum = ps_st.tile([1, F], F32, tag="ssum", name="ssum")
                    ssq = ps_st.tile([1, F], F32, tag="ssq", name="ssq")
                    for b in range(NBLK):
                        zsl = z_all[:, b * F:(b + 1) * F]
                        sq_t = wpool.tile([P, F], F16, tag="sq_t", name="sq_t",
                                          bufs=4)
                        nc.vector.tensor_tensor(out=sq_t[:], in0=zsl, in1=zsl,
                                                op=mybir.AluOpType.mult)
                        nc.tensor.matmul(out=ssum[:], lhsT=vmask[:], rhs=zsl,
                                         start=(b == 0), stop=(b == NBLK - 1))
                        nc.tensor.matmul(out=ssq[:], lhsT=vmask[:], rhs=sq_t[:],
                                         start=(b == 0), stop=(b == NBLK - 1))

                    # --- stats AllReduce + scale/shift ------------------------
                    srow = spool.tile([1, 2 * F], F32, tag="srow", name="srow")
                    nc.vector.tensor_copy(out=srow[:, 0:F], in_=ssum[:])
                    nc.vector.tensor_copy(out=srow[:, F:2 * F], in_=ssq[:])
                    nc.sync.dma_start(out=st_in[layer][:, :], in_=srow[:])
                    if DEBUG_NO_CC:
                        nc.sync.dma_start(out=st_out[layer][:, :],
                                          in_=st_in[layer][:, :])
                    else:
                        nc.gpsimd.collective_compute(
                            "AllReduce", mybir.AluOpType.add, replica_groups=rg,
                            ins=[st_in[layer][:, :]], outs=[st_out[layer][:, :]])
                    srow2 = spool.tile([1, 2 * F], F32, tag="srow2", name="srow2")
                    nc.sync.dma_start(out=srow2[:], in_=st_out[layer][:, :])

                    mu = spool.tile([1, F], F32, tag="mu", name="mu")
                    nc.vector.tensor_scalar(out=mu[:], in0=srow2[:, 0:F],
                                            scalar1=recip_n, scalar2=None,
                                            op0=mybir.AluOpType.mult)
                    ex2 = spool.tile([1, F], F32, tag="ex2", name="ex2")
                    nc.vector.tensor_scalar(out=ex2[:], in0=srow2[:, F:2 * F],
                                            scalar1=recip_n, scalar2=None,
                                            op0=mybir.AluOpType.mult)
                    var = spool.tile([1, F], F32, tag="var", name="var")
                    nc.vector.tensor_tensor(out=var[:], in0=mu[:], in1=mu[:],
                                            op=mybir.AluOpType.mult)
                    nc.vector.tensor_tensor(out=var[:], in0=ex2[:], in1=var[:],
                                            op=mybir.AluOpType.subtract)
                    sd = spool.tile([1, F], F32, tag="sd", name="sd")
                    nc.scalar.activation(out=sd[:], in_=var[:],
                                         func=mybir.ActivationFunctionType.Sqrt,
                                         bias=eps_t[:])
                    rstd = spool.tile([1, F], F32, tag="rstd", name="rstd")
                    nc.vector.reciprocal(rstd[:], sd[:])
                    s_row = spool.tile([1, F], F32, tag="s_row", name="s_row")
                    nc.vector.tensor_tensor(out=s_row[:], in0=rstd[:],
                                            in1=gb_t[layer][:, 0:F],
                                            op=mybir.AluOpType.mult)
                    t_row = spool.tile([1, F], F32, tag="t_row", name="t_row")
                    nc.vector.tensor_tensor(out=t_row[:], in0=mu[:], in1=s_row[:],
                                            op=mybir.AluOpType.mult)
                    nc.vector.tensor_tensor(out=t_row[:], in0=gb_t[layer][:, F:2 * F],
                                            in1=t_row[:],
                                            op=mybir.AluOpType.subtract)
                    S_b = spool.tile([P, F], F32, tag="S_b", name="S_b")
                    nc.gpsimd.partition_broadcast(out_ap=S_b[:], in_ap=s_row[:])
                    T_b = spool.tile([P, F], F32, tag="T_b", name="T_b")
                    nc.gpsimd.partition_broadcast(out_ap=T_b[:], in_ap=t_row[:])

                    # --- pass 2: h = relu(z*S + T), feed next stage ----------
                    if layer == 1:
                        pool_ps = ps_pool.tile([G, F], F32, tag="pool", name="pool")
                    for p in (0, 1):
                        for b in range(p * GBLK, (p + 1) * GBLK):
                            zsl = z_all[:, b * F:(b + 1) * F]
                            u = wpool.tile([P, F], F16, tag="u", name="u", bufs=4)
                            nc.vector.tensor_tensor(out=u[:], in0=zsl, in1=S_b[:],
                                                    op=mybir.AluOpType.mult)
                            u2 = wpool.tile([P, F], F16, tag="u2", name="u2", bufs=4)
                            nc.vector.tensor_tensor(out=u2[:], in0=u[:], in1=T_b[:],
                                                    op=mybir.AluOpType.add)
                            hp = wpool.tile([P, F], F16, tag="hp", name="hp")
                            if layer == 0:
                                nc.scalar.activation(
                                    out=hp[:], in_=u2[:],
                                    func=mybir.ActivationFunctionType.Relu,
                                    scale=ddst_t[:, b:b + 1])
                                for kc in (0, 1):
                                    tp = ps_misc.tile([P, P], F16, tag="misc", name="misc")
                                    nc.tensor.transpose(
                                        out=tp[:], in_=hp[:, kc * P:(kc + 1) * P],
                                        identity=ident[:])
                                    nc.vector.tensor_copy(
                                        out=hT2[kc][:, b * P:(b + 1) * P], in_=tp[:])
                                mp2 = ps_misc.tile([P, F], F32, tag="misc", name="misc")
                                for kc in (0, 1):
                                    nc.tensor.matmul(
                                        out=mp2[:], lhsT=hT2[kc][:, b * P:(b + 1) * P],
                                        rhs=w_t[1][kc][:],
                                        start=(kc == 0), stop=(kc == 1))
                                m_sb2 = wpool.tile([P, F], F16, tag="m_sb", name="m_sb")
                                nc.vector.tensor_copy(out=m_sb2[:], in_=mp2[:])
                                nc.sync.dma_start(
                                    out=ag_in[1][p][(b - p * GBLK) * NPB:
                                                    (b - p * GBLK + 1) * NPB, :],
                                    in_=m_sb2[0:NPB, :])
                            else:
                                nc.scalar.activation(
                                    out=hp[:], in_=u2[:],
                                    func=mybir.ActivationFunctionType.Relu)
                                pone = bpool.tile([P, G], F16, tag="pone", name="pone")
                                nc.vector.tensor_scalar(
                                    out=pone[:], in0=iota_f[:, 0:G],
                                    scalar1=bt_t[:, b:b + 1], scalar2=None,
                                    op0=mybir.AluOpType.is_equal)
                                nc.tensor.matmul(out=pool_ps[:], lhsT=pone[:],
                                                 rhs=hp[:], start=(b == 0),
                                                 stop=(b == NBLK - 1))
                        if layer == 0 and p == 0:
                            fire_ag(1, 0)

                pool_sb = spool.tile([G, F], F32, tag="pool_sb", name="pool_sb")
                nc.vector.tensor_copy(out=pool_sb[:], in_=pool_ps[:])
                nc.sync.dma_start(out=pool_out[:, :], in_=pool_sb[:])

    nc.compile()
    return nc


DEBUG_NO_CC = False   # replace collectives with local DMA (timing-only builds)

_CACHE = {}


def _get_program(cfg, NCH, SCP, NG, reps=1):
    key = (cfg.N, cfg.G, cfg.NBLK, cfg.NPB, NCH, SCP, NG, reps)
    if key not in _CACHE:
        _CACHE[key] = _build(cfg, NCH, SCP, NG, reps)
    return _CACHE[key]


def _run(inputs, cfg, trace=False):
    in_maps, cnt, NCH, SCP, NG = _preprocess(
        inputs["x"], inputs["ei"], inputs["batch"],
        inputs["W1"], inputs["g1"], inputs["be1"],
        inputs["W2"], inputs["g2"], inputs["be2"], cfg)
    nc = _get_program(cfg, NCH, SCP, NG)
    res = bass_utils.run_bass_kernel_spmd(
        nc, in_maps, core_ids=list(range(NCORES)), trace=trace)
    partial = np.zeros((cfg.G, F), np.float32)
    for c in range(NCORES):
        partial += np.asarray(res.results[c]["pool_out"], np.float32)
    out = partial / np.maximum(cnt, 1.0)[:, None]
    return out.astype(np.float32), res


def kernel(**inputs):
    cfg = Cfg(N=40000, G=64, NBLK=40, NPB=125)
    out, _ = _run(inputs, cfg)
    return out


# revision 8
# speedup vs baseline: 2.2459x; 1.0442x over previous
"""Trainium2 Bass kernel for a 2-layer GCN encoder (40000 nodes, 640000 edges,
256 features, 64-graph mean pooling), SPMD across 8 NeuronCores.

Strategy
--------
Math: per layer  z = dinv * Agg(m') ,  m' = dinv * (input @ W)  (self-loops in
the edge list; the bias cancels inside training-mode BatchNorm), then
h = relu(bn(z)).  Output = per-graph mean pool of layer-2 h.

Sharding: nodes are sharded contiguously across the 8 cores (5000 each).  On
each core its 5000 destination nodes are permuted into 40 blocks of 125; the
blocks are split into two GROUPS (0-19, 20-39).  The per-layer node-feature
table is AllGathered in TWO pieces (one per group), so the group-0 collective
fires as soon as the first half of the previous layer's pass-2 finishes, and
aggregation over group-0 sources overlaps the group-1 collective.

Aggregation runs as two sweeps over all 40 dst blocks (one per source part);
each (block, part) contributes NCH padded 128-edge chunks, accumulated in
PSUM by TensorEngine matmuls
   psum[dst 128, feat 256] += onehot[edge 128, dst 128]^T @ gathered[edge 128, feat 256]
with the fp8 one-hot streamed from DRAM and source rows fetched by
`dma_gather` (1024 rows / instruction, 4 SWDGE queues) from the part's
AllGathered fp16 table.  Sweep results are combined in SBUF (fp16) with the
dinv_dst scale applied on the Scalar engine (activation Copy), keeping the
per-block tensor queue free of cross-engine stalls.  BatchNorm statistics are
computed in a separate end-of-layer sweep (ones-mask matmuls), AllReduced,
and applied in pass 2, whose group-0 half feeds the next layer's group-0
AllGather immediately.  Greedy two-iteration balancing assigns nodes to
blocks so per-(block, part) edge counts stay under NCH*128 on every core.
"""

import numpy as np
import ml_dtypes

import concourse.bacc as bacc
import concourse.bass as bass
import concourse.mybir as mybir
import concourse.tile as tile
from concourse import bass_utils

P = 128
F = 256          # feature width (both layers)
NCORES = 8
BN_EPS = 1e-5
NI = 1024        # rows per dma_gather
CPG = NI // P    # chunks per gather = 8

BF16 = mybir.dt.bfloat16
F16 = mybir.dt.float16
F32 = mybir.dt.float32
I16 = mybir.dt.int16
I32 = mybir.dt.int32


class Cfg:
    def __init__(self, N, G, NBLK, NPB):
        assert N == NCORES * NBLK * NPB and NPB <= P
        assert NBLK % 2 == 0
        self.N, self.G, self.NBLK, self.NPB = N, G, NBLK, NPB
        self.NPC = NBLK * NPB          # nodes per core
        self.GBLK = NBLK // 2          # blocks per group
        self.PHALF = N // 2            # rows per part table


# ----------------------------------------------------------------------------
# host-side preprocessing
# ----------------------------------------------------------------------------

def _preprocess(x, ei, batch, W1, g1, be1, W2, g2, be2, cfg):
    N, G, NBLK, NPB, NPC = cfg.N, cfg.G, cfg.NBLK, cfg.NPB, cfg.NPC
    GBLK, PHALF = cfg.GBLK, cfg.PHALF
    PNPC = NPC // 2

    loops = np.arange(N, dtype=np.int64)
    src = np.asarray(ei[0], dtype=np.int64)
    dst = np.asarray(ei[1], dtype=np.int64)

    # degree includes the self-loop; the loop's contribution itself is added
    # on-device from the resident transposed tables (two matmuls per block)
    deg = (np.bincount(dst, minlength=N) + 1).astype(np.float64)
    dinv = (1.0 / np.sqrt(deg)).astype(np.float32)

    node_core = (np.arange(N) // NPC).astype(np.int32)

    # phase 1: pin each node's part (source-side group) up front — per core,
    # alternate by descending degree so both parts carry similar total
    # out-degree; parts then stay FIXED, so dst-side per-part in-degrees are
    # known exactly before block assignment.
    degs = np.bincount(src, minlength=N)          # out-degree (incl. loop)
    part = np.empty(N, np.int64)
    for c in range(NCORES):
        nodes = np.arange(c * NPC, (c + 1) * NPC)
        order = nodes[np.argsort(-degs[nodes], kind="stable")]
        part[order[0::2]] = 0
        part[order[1::2]] = 1

    m0 = part[src] == 0
    deg0 = np.bincount(dst[m0], minlength=N)
    deg1 = np.bincount(dst[~m0], minlength=N)

    # phase 2: per core and per group, greedily assign that group's 2500
    # nodes to its 20 blocks, balancing both per-part in-degree loads.
    blk = np.empty(N, np.int32)
    slot = np.empty(N, np.int32)
    for c in range(NCORES):
        for p in (0, 1):
            nodes = np.arange(c * NPC, (c + 1) * NPC)
            nodes = nodes[part[nodes] == p]
            order = nodes[np.argsort(-(deg0[nodes] + deg1[nodes]),
                                     kind="stable")]
            loadA = np.zeros(GBLK, np.int64)
            loadB = np.zeros(GBLK, np.int64)
            cnt_b = np.zeros(GBLK, np.int64)
            for n in order:
                score = np.maximum(loadA + deg0[n], loadB + deg1[n]).astype(
                    np.float64)
                score[cnt_b >= NPB] = np.inf
                b = int(np.argmin(score))
                blk[n] = p * GBLK + b
                slot[n] = cnt_b[b]
                cnt_b[b] += 1
                loadA[b] += deg0[n]
                loadB[b] += deg1[n]

    # local swap repair: push per-(block, part) loads under 8*P if possible
    degP = np.stack([deg0, deg1], axis=1)
    L = np.zeros((NCORES * NBLK, 2), np.int64)
    for p in (0, 1):
        np.add.at(L[:, p], node_core * NBLK + blk, degP[:, p])
    gb = node_core * NBLK + blk
    order_m = np.argsort(gb, kind="stable")
    bounds = np.searchsorted(gb[order_m], np.arange(NCORES * NBLK + 1))
    members = {cb: list(order_m[bounds[cb]:bounds[cb + 1]])
               for cb in range(NCORES * NBLK)}
    target = 8 * P
    for _ in range(3000):
        worst = int(np.argmax(L.max(axis=1)))
        if L[worst].max() <= target:
            break
        p_hot = int(np.argmax(L[worst]))
        c = worst // NBLK
        grp = (worst % NBLK) // GBLK
        best = None
        mem_w = sorted(members[worst], key=lambda n: -degP[n, p_hot])[:12]
        for j in range(GBLK):
            b2 = c * NBLK + grp * GBLK + j
            if b2 == worst:
                continue
            mem2 = sorted(members[b2], key=lambda n: degP[n, p_hot])[:12]
            for n in mem_w:
                for m in mem2:
                    nm = max(L[worst][0] - degP[n, 0] + degP[m, 0],
                             L[worst][1] - degP[n, 1] + degP[m, 1],
                             L[b2][0] + degP[n, 0] - degP[m, 0],
                             L[b2][1] + degP[n, 1] - degP[m, 1])
                    if best is None or nm < best[0]:
                        best = (nm, n, m, b2)
        if best is None or best[0] >= L[worst].max():
            break
        _, n, m, b2 = best
        members[worst].remove(n); members[b2].remove(m)
        members[worst].append(m); members[b2].append(n)
        for p in (0, 1):
            L[worst][p] += degP[m, p] - degP[n, p]
            L[b2][p] += degP[n, p] - degP[m, p]
        blk[n], blk[m] = blk[m], blk[n]
        slot[n], slot[m] = slot[m], slot[n]

    # row of each node inside its part's AllGather table
    agrow_p = node_core * PNPC + (blk - part * GBLK) * NPB + slot

    # group edges by (dst core, dst block, src part)
    ecore = (dst // NPC).astype(np.int32)
    eblk = blk[dst]
    edstl = slot[dst]
    epart = part[src].astype(np.int32)
    key = (ecore * NBLK + eblk) * 2 + epart
    order = np.lexsort((src, key))
    s_key = key[order]
    s_src = src[order]
    s_dstl = edstl[order].astype(np.float32)

    counts = np.bincount(key, minlength=NCORES * NBLK * 2)
    NCH = int(np.ceil(counts.max() / P))          # chunks per (block, part)
    SC = NBLK * NCH                               # stream chunks per part
    SCP = ((SC + CPG - 1) // CPG) * CPG           # padded to gather multiple
    NG = SCP // CPG                               # gathers per part-stream

    starts = np.concatenate([[0], np.cumsum(counts)])[:-1]
    rank = np.arange(len(s_key)) - starts[s_key]
    chunkrel = rank // P
    spart = s_key % 2
    score_blk = (s_key // 2) % NBLK               # dst block
    score_core = s_key // (2 * NBLK)              # dst core
    scol = score_blk * NCH + chunkrel             # stream chunk column
    flat = scol * P + (rank % P)                  # position within stream

    hrow = agrow_p[s_src].astype(np.int16)
    valid_f = np.ones(len(s_src), np.float32)

    in_maps = []
    xp = np.asarray(x, dtype=np.float32) * dinv[:, None]
    W1b = np.asarray(W1, dtype=np.float32).reshape(2, P, F).astype(np.float16)
    W2b = np.asarray(W2, dtype=np.float32).reshape(2, P, F).astype(np.float16)
    gb1 = np.concatenate([np.asarray(g1, np.float32),
                          np.asarray(be1, np.float32)])[None, :]
    gb2 = np.concatenate([np.asarray(g2, np.float32),
                          np.asarray(be2, np.float32)])[None, :]
    batch = np.asarray(batch, dtype=np.int64)

    for c in range(NCORES):
        m = {}
        for p in (0, 1):
            sel = (score_core == c) & (spart == p)
            vidx = np.zeros(SCP * P, np.int16)
            vdstl = np.zeros(SCP * P, np.float32)
            vvalid = np.zeros(SCP * P, np.float32)
            f = flat[sel]
            vidx[f] = hrow[sel]
            vdstl[f] = s_dstl[sel]
            vvalid[f] = valid_f[sel]
            # wrap idxs: idx i -> [i%16, i//16], replicated to 128 partitions
            w16 = vidx.reshape(-1, 16).T               # [16, SCP*8]
            m[f"idx{p}"] = np.ascontiguousarray(np.tile(w16, (8, 1)))
            # fp8 one-hot blob: Bb[q, scol*128 + d] = (dstl==d) & valid
            dstl2 = vdstl.reshape(SCP, P).T            # [128, SCP]
            valid = (vvalid.reshape(SCP, P).T != 0.0)
            oneh = (dstl2[:, :, None] ==
                    np.arange(P, dtype=np.float32)[None, None, :]) & valid[:, :, None]
            m[f"bb{p}"] = np.ascontiguousarray(
                oneh.reshape(P, SCP * P)).astype(ml_dtypes.float8_e4m3)

        nodes = np.arange(c * NPC, (c + 1) * NPC)
        col = blk[nodes] * P + slot[nodes]
        ddst = np.zeros((NBLK * P,), np.float32)
        ddst[col] = dinv[nodes]
        m["ddst"] = np.ascontiguousarray(ddst.reshape(NBLK, P).T)   # [128, NBLK]
        bt = np.full((NBLK * P,), 1000.0, np.float32)
        bt[col] = batch[nodes].astype(np.float32)
        m["bt"] = np.ascontiguousarray(bt.reshape(NBLK, P).T)       # [128, NBLK]

        xa = np.zeros((NBLK * P, F), np.float32)
        xa[col] = xp[nodes]
        m["xT"] = np.ascontiguousarray(
            xa.T.reshape(2, P, NBLK * P)).astype(np.float16)

        m["w1"] = W1b
        m["w2"] = W2b
        m["gb1"] = gb1
        m["gb2"] = gb2
        in_maps.append(m)

    cnt = np.bincount(batch, minlength=G).astype(np.float32)
    return in_maps, cnt, NCH, SCP, NG


# ----------------------------------------------------------------------------
# device program
# ----------------------------------------------------------------------------

def _build(cfg, NCH, SCP, NG, reps=1):
    N, G, NBLK, NPB, NPC = cfg.N, cfg.G, cfg.NBLK, cfg.NPB, cfg.NPC
    GBLK, PHALF = cfg.GBLK, cfg.PHALF
    rg = [list(range(NCORES))]

    nc = bacc.Bacc("TRN2", target_bir_lowering=False, debug=False,
                   num_devices=NCORES, num_swdge_queues=4)

    F8 = mybir.dt.float8e4
    din = {}
    for p in (0, 1):
        din[f"idx{p}"] = nc.dram_tensor(f"idx{p}", [P, SCP * 8], I16,
                                        kind="ExternalInput")
        din[f"bb{p}"] = nc.dram_tensor(f"bb{p}", [P, SCP * P], F8,
                                       kind="ExternalInput")
    din["ddst"] = nc.dram_tensor("ddst", [P, NBLK], F32, kind="ExternalInput")
    din["bt"] = nc.dram_tensor("bt", [P, NBLK], F32, kind="ExternalInput")
    din["xT"] = nc.dram_tensor("xT", [2, P, NBLK * P], F16, kind="ExternalInput")
    din["w1"] = nc.dram_tensor("w1", [2, P, F], F16, kind="ExternalInput")
    din["w2"] = nc.dram_tensor("w2", [2, P, F], F16, kind="ExternalInput")
    din["gb1"] = nc.dram_tensor("gb1", [1, 2 * F], F32, kind="ExternalInput")
    din["gb2"] = nc.dram_tensor("gb2", [1, 2 * F], F32, kind="ExternalInput")

    pool_out = nc.dram_tensor("pool_out", [G, F], F32, kind="ExternalOutput")

    ag_in = [[nc.dram_tensor(f"ag_in{l}_{p}", [NPC // 2, F], F16,
                             kind="Internal") for p in (0, 1)] for l in (0, 1)]
    ag_out = [[nc.dram_tensor(f"ag_out{l}_{p}", [PHALF, F], F16,
                              kind="Internal", addr_space="Shared")
               for p in (0, 1)] for l in (0, 1)]
    st_in = [nc.dram_tensor(f"st_in{l}", [1, 2 * F], F32, kind="Internal")
             for l in (0, 1)]
    st_out = [nc.dram_tensor(f"st_out{l}", [1, 2 * F], F32, kind="Internal",
                             addr_space="Shared") for l in (0, 1)]

    with tile.TileContext(nc) as tc:
        import contextlib
        with contextlib.ExitStack() as ctx:
            meta = ctx.enter_context(tc.tile_pool(name="meta", bufs=1))
            big = ctx.enter_context(tc.tile_pool(name="big", bufs=1))
            gpools = [ctx.enter_context(tc.tile_pool(name=f"g{p}", bufs=9))
                      for p in (0, 1)]
            bpool = ctx.enter_context(tc.tile_pool(name="bpool", bufs=8))
            wpool = ctx.enter_context(tc.tile_pool(name="wpool", bufs=3))
            spool = ctx.enter_context(tc.tile_pool(name="spool", bufs=2))
            ps_agg = ctx.enter_context(
                tc.tile_pool(name="ps_agg", bufs=3, space="PSUM"))
            ps_st = ctx.enter_context(
                tc.tile_pool(name="ps_st", bufs=1, space="PSUM"))
            ps_misc = ctx.enter_context(
                tc.tile_pool(name="ps_misc", bufs=2, space="PSUM"))
            ps_pool = ctx.enter_context(
                tc.tile_pool(name="ps_pool", bufs=1, space="PSUM"))

            # --- resident data
            hT1 = [big.tile([P, NBLK * P], F16, tag=f"hT1_{kc}", name=f"hT1_{kc}")
                   for kc in (0, 1)]
            for kc in (0, 1):
                nc.sync.dma_start(out=hT1[kc][:], in_=din["xT"][kc, :, :])
            w_t = []
            for l, name in ((0, "w1"), (1, "w2")):
                tiles = []
                for kc in (0, 1):
                    wt = meta.tile([P, F], F16, tag=f"{name}_{kc}", name=f"{name}_{kc}")
                    nc.sync.dma_start(out=wt[:], in_=din[name][kc, :, :])
                    tiles.append(wt)
                w_t.append(tiles)
            idx_t = []
            for p in (0, 1):
                it = meta.tile([P, SCP * 8], I16, tag=f"idx{p}", name=f"idx{p}")
                nc.sync.dma_start(out=it[:], in_=din[f"idx{p}"][:, :])
                idx_t.append(it)
            ddst_t = meta.tile([P, NBLK], F32, tag="ddst", name="ddst")
            nc.sync.dma_start(out=ddst_t[:], in_=din["ddst"][:, :])
            bt_t = meta.tile([P, NBLK], F32, tag="bt", name="bt")
            nc.sync.dma_start(out=bt_t[:], in_=din["bt"][:, :])

            gb_t = []
            for l, name in ((0, "gb1"), (1, "gb2")):
                gt = meta.tile([1, 2 * F], F32, tag=name, name=name)
                nc.sync.dma_start(out=gt[:], in_=din[name][:, :])
                gb_t.append(gt)

            iota_i = meta.tile([P, P], I32, tag="iota_i", name="iota_i")
            nc.gpsimd.iota(iota_i[:], [[1, P]], channel_multiplier=0)
            iota_f = meta.tile([P, P], F32, tag="iota_f", name="iota_f")
            nc.vector.tensor_copy(out=iota_f[:], in_=iota_i[:])

            from concourse.masks import make_identity
            ident = meta.tile([P, P], F16, tag="ident", name="ident")
            make_identity(nc, ident[:])

            vmask = meta.tile([P, 1], F16, tag="vmask", name="vmask")
            nc.vector.memset(vmask[:], 0.0)
            nc.vector.memset(vmask[0:NPB, :], 1.0)

            eps_t = meta.tile([1, 1], F32, tag="eps_t", name="eps_t")
            nc.vector.memset(eps_t[:], BN_EPS)

            hT2 = [big.tile([P, NBLK * P], F16, tag=f"hT2_{kc}", name=f"hT2_{kc}")
                   for kc in (0, 1)]
            z_all = big.tile([P, NBLK * F], F16, tag="z_all", name="z_all")

            recip_n = 1.0 / float(N)

            def fire_ag(layer, p):
                if DEBUG_NO_CC:
                    nc.sync.dma_start(
                        out=ag_out[layer][p][0:NPC // 2, :],
                        in_=ag_in[layer][p][:, :])
                else:
                    nc.gpsimd.collective_compute(
                        "AllGather", mybir.AluOpType.bypass, replica_groups=rg,
                        ins=[ag_in[layer][p][:, :]],
                        outs=[ag_out[layer][p][:, :]])

            for rep in range(reps):
                # --- layer-0 table: per-group matmuls + early AllGathers ----
                for p in (0, 1):
                    for b in range(p * GBLK, (p + 1) * GBLK):
                        mp = ps_misc.tile([P, F], F32, tag="misc", name="misc")
                        for kc in (0, 1):
                            nc.tensor.matmul(
                                out=mp[:], lhsT=hT1[kc][:, b * P:(b + 1) * P],
                                rhs=w_t[0][kc][:],
                                start=(kc == 0), stop=(kc == 1))
                        m_sb = wpool.tile([P, F], F16, tag="m_sb", name="m_sb")
                        nc.vector.tensor_copy(out=m_sb[:], in_=mp[:])
                        nc.sync.dma_start(
                            out=ag_in[0][p][(b - p * GBLK) * NPB:
                                            (b - p * GBLK + 1) * NPB, :],
                            in_=m_sb[0:NPB, :])
                    fire_ag(0, p)

                for layer in (0, 1):
                    # --- aggregation: one sweep per source part -------------
                    gtiles = {0: {}, 1: {}}
                    btiles = {0: {}, 1: {}}

                    def ensure_gather(p, gi, layer=layer, gtiles=gtiles):
                        if gi in gtiles[p]:
                            return gtiles[p][gi]
                        gt = gpools[p].tile([P, CPG * F], F16, tag=f"gt{p}",
                                            name=f"gt{p}")
                        nc.gpsimd.dma_gather(
                            out_ap=gt[:].rearrange("p (c d) -> p c d", d=F),
                            in_ap=ag_out[layer][p][:, :],
                            idxs_ap=idx_t[p][:, gi * (NI // 16):(gi + 1) * (NI // 16)],
                            num_idxs=NI, num_idxs_reg=NI, elem_size=F,
                            queue_num=gi % 4)
                        gtiles[p][gi] = gt
                        return gt

                    def ensure_btile(p, gi, btiles=btiles):
                        if gi in btiles[p]:
                            return btiles[p][gi]
                        bt_ = bpool.tile([P, CPG * P], F8, tag=f"bb{p}",
                                         name=f"bb{p}")
                        nc.sync.dma_start(
                            out=bt_[:],
                            in_=din[f"bb{p}"][:, gi * CPG * P:(gi + 1) * CPG * P])
                        btiles[p][gi] = bt_
                        return bt_

                    hTl = hT1 if layer == 0 else hT2
                    for p in (0, 1):
                        if p == 1 and layer == 1:
                            fire_ag(layer, 1)
                        for b in range(NBLK):
                            agg = ps_agg.tile([P, F], F32, tag="agg", name="agg")
                            if p == 0:
                                # self-loop term: this block's own table rows,
                                # recomputed from the resident transposed input
                                for kc in (0, 1):
                                    nc.tensor.matmul(
                                        out=agg[:],
                                        lhsT=hTl[kc][:, b * P:(b + 1) * P],
                                        rhs=w_t[layer][kc][:],
                                        start=(kc == 0), stop=False)
                            for j in range(NCH):
                                scol = b * NCH + j
                                gi, gslot = divmod(scol, CPG)
                                gt = ensure_gather(p, gi)
                                bt_ = ensure_btile(p, gi)
                                nc.tensor.matmul(
                                    out=agg[:],
                                    lhsT=bt_[:, gslot * P:(gslot + 1) * P],
                                    rhs=gt[:, gslot * F:(gslot + 1) * F],
                                    start=(p == 0 and False) or (p == 1 and j == 0),
                                    stop=(j == NCH - 1))
                            zsl = z_all[:, b * F:(b + 1) * F]
                            if p == 0:
                                nc.scalar.activation(
                                    out=zsl, in_=agg[:],
                                    func=mybir.ActivationFunctionType.Copy,
                                    scale=ddst_t[:, b:b + 1])
                            else:
                                zt = wpool.tile([P, F], F16, tag="zt",
                                                name="zt", bufs=4)
                                nc.scalar.activation(
                                    out=zt[:], in_=agg[:],
                                    func=mybir.ActivationFunctionType.Copy,
                                    scale=ddst_t[:, b:b + 1])
                                nc.vector.tensor_tensor(
                                    out=zsl, in0=zt[:], in1=zsl,
                                    op=mybir.AluOpType.add)

                    # --- BN stats: end-of-layer sweep ------------------------
                    ssum = ps_st.tile([1, F], F32, tag="ssum", name="ssum")
                    ssq = ps_st.tile([1, F], F32, tag="ssq", name="ssq")
                    for b in range(NBLK):
                        zsl = z_all[:, b * F:(b + 1) * F]
                        sq_t = wpool.tile([P, F], F16, tag="sq_t", name="sq_t",
                                          bufs=4)
                        nc.vector.tensor_tensor(out=sq_t[:], in0=zsl, in1=zsl,
                                                op=mybir.AluOpType.mult)
                        nc.tensor.matmul(out=ssum[:], lhsT=vmask[:], rhs=zsl,
                                         start=(b == 0), stop=(b == NBLK - 1))
                        nc.tensor.matmul(out=ssq[:], lhsT=vmask[:], rhs=sq_t[:],
                                         start=(b == 0), stop=(b == NBLK - 1))

                    # --- stats AllReduce + scale/shift ------------------------
                    srow = spool.tile([1, 2 * F], F32, tag="srow", name="srow")
                    nc.vector.tensor_copy(out=srow[:, 0:F], in_=ssum[:])
                    nc.vector.tensor_copy(out=srow[:, F:2 * F], in_=ssq[:])
                    nc.sync.dma_start(out=st_in[layer][:, :], in_=srow[:])
                    if DEBUG_NO_CC:
                        nc.sync.dma_start(out=st_out[layer][:, :],
                                          in_=st_in[layer][:, :])
                    else:
                        nc.gpsimd.collective_compute(
                            "AllReduce", mybir.AluOpType.add, replica_groups=rg,
                            ins=[st_in[layer][:, :]], outs=[st_out[layer][:, :]])
                    srow2 = spool.tile([1, 2 * F], F32, tag="srow2", name="srow2")
                    nc.sync.dma_start(out=srow2[:], in_=st_out[layer][:, :])

                    mu = spool.tile([1, F], F32, tag="mu", name="mu")
                    nc.vector.tensor_scalar(out=mu[:], in0=srow2[:, 0:F],
                                            scalar1=recip_n, scalar2=None,
                                            op0=mybir.AluOpType.mult)
                    ex2 = spool.tile([1, F], F32, tag="ex2", name="ex2")
                    nc.vector.tensor_scalar(out=ex2[:], in0=srow2[:, F:2 * F],
                                            scalar1=recip_n, scalar2=None,
                                            op0=mybir.AluOpType.mult)
                    var = spool.tile([1, F], F32, tag="var", name="var")
                    nc.vector.tensor_tensor(out=var[:], in0=mu[:], in1=mu[:],
                                            op=mybir.AluOpType.mult)
                    nc.vector.tensor_tensor(out=var[:], in0=ex2[:], in1=var[:],
                                            op=mybir.AluOpType.subtract)
                    sd = spool.tile([1, F], F32, tag="sd", name="sd")
                    nc.scalar.activation(out=sd[:], in_=var[:],
                                         func=mybir.ActivationFunctionType.Sqrt,
                                         bias=eps_t[:])
                    rstd = spool.tile([1, F], F32, tag="rstd", name="rstd")
                    nc.vector.reciprocal(rstd[:], sd[:])
                    s_row = spool.tile([1, F], F32, tag="s_row", name="s_row")
                    nc.vector.tensor_tensor(out=s_row[:], in0=rstd[:],
                                            in1=gb_t[layer][:, 0:F],
                                            op=mybir.AluOpType.mult)
                    t_row = spool.tile([1, F], F32, tag="t_row", name="t_row")
                    nc.vector.tensor_tensor(out=t_row[:], in0=mu[:], in1=s_row[:],
                                            op=mybir.AluOpType.mult)
                    nc.vector.tensor_tensor(out=t_row[:], in0=gb_t[layer][:, F:2 * F],
                                            in1=t_row[:],
                                            op=mybir.AluOpType.subtract)
                    S_b = spool.tile([P, F], F32, tag="S_b", name="S_b")
                    nc.gpsimd.partition_broadcast(out_ap=S_b[:], in_ap=s_row[:])
                    T_b = spool.tile([P, F], F32, tag="T_b", name="T_b")
                    nc.gpsimd.partition_broadcast(out_ap=T_b[:], in_ap=t_row[:])

                    # --- pass 2: h = relu(z*S + T), feed next stage ----------
                    if layer == 1:
                        pool_ps = ps_pool.tile([G, F], F32, tag="pool", name="pool")
                    for p in (0, 1):
                        for b in range(p * GBLK, (p + 1) * GBLK):
                            zsl = z_all[:, b * F:(b + 1) * F]
                            u = wpool.tile([P, F], F16, tag="u", name="u", bufs=4)
                            nc.vector.tensor_tensor(out=u[:], in0=zsl, in1=S_b[:],
                                                    op=mybir.AluOpType.mult)
                            u2 = wpool.tile([P, F], F16, tag="u2", name="u2", bufs=4)
                            nc.vector.tensor_tensor(out=u2[:], in0=u[:], in1=T_b[:],
                                                    op=mybir.AluOpType.add)
                            hp = wpool.tile([P, F], F16, tag="hp", name="hp")
                            if layer == 0:
                                nc.scalar.activation(
                                    out=hp[:], in_=u2[:],
                                    func=mybir.ActivationFunctionType.Relu,
                                    scale=ddst_t[:, b:b + 1])
                                for kc in (0, 1):
                                    tp = ps_misc.tile([P, P], F16, tag="misc", name="misc")
                                    nc.tensor.transpose(
                                        out=tp[:], in_=hp[:, kc * P:(kc + 1) * P],
                                        identity=ident[:])
                                    nc.vector.tensor_copy(
                                        out=hT2[kc][:, b * P:(b + 1) * P], in_=tp[:])
                                mp2 = ps_misc.tile([P, F], F32, tag="misc", name="misc")
                                for kc in (0, 1):
                                    nc.tensor.matmul(
                                        out=mp2[:], lhsT=hT2[kc][:, b * P:(b + 1) * P],
                                        rhs=w_t[1][kc][:],
                                        start=(kc == 0), stop=(kc == 1))
                                m_sb2 = wpool.tile([P, F], F16, tag="m_sb", name="m_sb")
                                nc.vector.tensor_copy(out=m_sb2[:], in_=mp2[:])
                                nc.sync.dma_start(
                                    out=ag_in[1][p][(b - p * GBLK) * NPB:
                                                    (b - p * GBLK + 1) * NPB, :],
                                    in_=m_sb2[0:NPB, :])
                            else:
                                nc.scalar.activation(
                                    out=hp[:], in_=u2[:],
                                    func=mybir.ActivationFunctionType.Relu)
                                pone = bpool.tile([P, G], F16, tag="pone", name="pone")
                                nc.vector.tensor_scalar(
                                    out=pone[:], in0=iota_f[:, 0:G],
                                    scalar1=bt_t[:, b:b + 1], scalar2=None,
                                    op0=mybir.AluOpType.is_equal)
                                nc.tensor.matmul(out=pool_ps[:], lhsT=pone[:],
                                                 rhs=hp[:], start=(b == 0),
                                                 stop=(b == NBLK - 1))
                        if layer == 0 and p == 0:
                            fire_ag(1, 0)

                pool_sb = spool.tile([G, F], F32, tag="pool_sb", name="pool_sb")
                nc.vector.tensor_copy(out=pool_sb[:], in_=pool_ps[:])
                nc.sync.dma_start(out=pool_out[:, :], in_=pool_sb[:])

    nc.compile()
    return nc


DEBUG_NO_CC = False   # replace collectives with local DMA (timing-only builds)

_CACHE = {}


def _get_program(cfg, NCH, SCP, NG, reps=1):
    key = (cfg.N, cfg.G, cfg.NBLK, cfg.NPB, NCH, SCP, NG, reps)
    if key not in _CACHE:
        _CACHE[key] = _build(cfg, NCH, SCP, NG, reps)
    return _CACHE[key]


def _run(inputs, cfg, trace=False):
    in_maps, cnt, NCH, SCP, NG = _preprocess(
        inputs["x"], inputs["ei"], inputs["batch"],
        inputs["W1"], inputs["g1"], inputs["be1"],
        inputs["W2"], inputs["g2"], inputs["be2"], cfg)
    nc = _get_program(cfg, NCH, SCP, NG)
    res = bass_utils.run_bass_kernel_spmd(
        nc, in_maps, core_ids=list(range(NCORES)), trace=trace)
    partial = np.zeros((cfg.G, F), np.float32)
    for c in range(NCORES):
        partial += np.asarray(res.results[c]["pool_out"], np.float32)
    out = partial / np.maximum(cnt, 1.0)[:, None]
    return out.astype(np.float32), res


def kernel(**inputs):
    cfg = Cfg(N=40000, G=64, NBLK=40, NPB=125)
    out, _ = _run(inputs, cfg)
    return out


# revision 9
# speedup vs baseline: 2.3942x; 1.0660x over previous
"""Trainium2 Bass kernel for a 2-layer GCN encoder (40000 nodes, 640000 edges,
256 features, 64-graph mean pooling), SPMD across 8 NeuronCores.

Strategy
--------
Math: per layer  z = dinv * Agg(m') ,  m' = dinv * (input @ W)  (self-loops in
the edge list; the bias cancels inside training-mode BatchNorm), then
h = relu(bn(z)).  Output = per-graph mean pool of layer-2 h.

Sharding: nodes are sharded contiguously across the 8 cores (5000 each).  On
each core its 5000 destination nodes are permuted into 40 blocks of 125; the
blocks are split into two GROUPS (0-19, 20-39).  The per-layer node-feature
table is AllGathered in TWO pieces (one per group), so the group-0 collective
fires as soon as the first half of the previous layer's pass-2 finishes, and
aggregation over group-0 sources overlaps the group-1 collective.

Aggregation runs as two sweeps over all 40 dst blocks (one per source part);
each (block, part) contributes NCH padded 128-edge chunks, accumulated in
PSUM by TensorEngine matmuls
   psum[dst 128, feat 256] += onehot[edge 128, dst 128]^T @ gathered[edge 128, feat 256]
with the fp8 one-hot streamed from DRAM and source rows fetched by
`dma_gather` (1024 rows / instruction, 4 SWDGE queues) from the part's
AllGathered fp16 table.  Sweep results are combined in SBUF (fp16) with the
dinv_dst scale applied on the Scalar engine (activation Copy), keeping the
per-block tensor queue free of cross-engine stalls.  BatchNorm statistics are
computed in a separate end-of-layer sweep (ones-mask matmuls), AllReduced,
and applied in pass 2, whose group-0 half feeds the next layer's group-0
AllGather immediately.  Greedy two-iteration balancing assigns nodes to
blocks so per-(block, part) edge counts stay under NCH*128 on every core.
"""

import numpy as np
import ml_dtypes

import concourse.bacc as bacc
import concourse.bass as bass
import concourse.mybir as mybir
import concourse.tile as tile
from concourse import bass_utils

P = 128
F = 256          # feature width (both layers)
NCORES = 8
BN_EPS = 1e-5
NI = 1024        # rows per dma_gather
CPG = NI // P    # chunks per gather = 8

BF16 = mybir.dt.bfloat16
F16 = mybir.dt.float16
F32 = mybir.dt.float32
I16 = mybir.dt.int16
I32 = mybir.dt.int32


class Cfg:
    def __init__(self, N, G, NBLK, NPB):
        assert N == NCORES * NBLK * NPB and NPB <= P
        assert NBLK % 2 == 0
        self.N, self.G, self.NBLK, self.NPB = N, G, NBLK, NPB
        self.NPC = NBLK * NPB          # nodes per core
        self.GBLK = NBLK // 2          # blocks per group
        self.PHALF = N // 2            # rows per part table


# ----------------------------------------------------------------------------
# host-side preprocessing
# ----------------------------------------------------------------------------

def _preprocess(x, ei, batch, W1, g1, be1, W2, g2, be2, cfg):
    N, G, NBLK, NPB, NPC = cfg.N, cfg.G, cfg.NBLK, cfg.NPB, cfg.NPC
    GBLK, PHALF = cfg.GBLK, cfg.PHALF
    PNPC = NPC // 2

    loops = np.arange(N, dtype=np.int64)
    src = np.asarray(ei[0], dtype=np.int64)
    dst = np.asarray(ei[1], dtype=np.int64)

    # degree includes the self-loop; the loop's contribution itself is added
    # on-device from the resident transposed tables (two matmuls per block)
    deg = (np.bincount(dst, minlength=N) + 1).astype(np.float64)
    dinv = (1.0 / np.sqrt(deg)).astype(np.float32)

    node_core = (np.arange(N) // NPC).astype(np.int32)

    # phase 1: pin each node's part (source-side group) up front — per core,
    # alternate by descending degree so both parts carry similar total
    # out-degree; parts then stay FIXED, so dst-side per-part in-degrees are
    # known exactly before block assignment.
    degs = np.bincount(src, minlength=N)          # out-degree (incl. loop)
    part = np.empty(N, np.int64)
    for c in range(NCORES):
        nodes = np.arange(c * NPC, (c + 1) * NPC)
        order = nodes[np.argsort(-degs[nodes], kind="stable")]
        part[order[0::2]] = 0
        part[order[1::2]] = 1

    m0 = part[src] == 0
    deg0 = np.bincount(dst[m0], minlength=N)
    deg1 = np.bincount(dst[~m0], minlength=N)

    # phase 2: per core and per group, greedily assign that group's 2500
    # nodes to its 20 blocks, balancing both per-part in-degree loads.
    blk = np.empty(N, np.int32)
    slot = np.empty(N, np.int32)
    for c in range(NCORES):
        for p in (0, 1):
            nodes = np.arange(c * NPC, (c + 1) * NPC)
            nodes = nodes[part[nodes] == p]
            order = nodes[np.argsort(-(deg0[nodes] + deg1[nodes]),
                                     kind="stable")]
            loadA = np.zeros(GBLK, np.int64)
            loadB = np.zeros(GBLK, np.int64)
            cnt_b = np.zeros(GBLK, np.int64)
            for n in order:
                score = np.maximum(loadA + deg0[n], loadB + deg1[n]).astype(
                    np.float64)
                score[cnt_b >= NPB] = np.inf
                b = int(np.argmin(score))
                blk[n] = p * GBLK + b
                slot[n] = cnt_b[b]
                cnt_b[b] += 1
                loadA[b] += deg0[n]
                loadB[b] += deg1[n]

    # local swap repair: push per-(block, part) loads under 8*P if possible
    degP = np.stack([deg0, deg1], axis=1)
    L = np.zeros((NCORES * NBLK, 2), np.int64)
    for p in (0, 1):
        np.add.at(L[:, p], node_core * NBLK + blk, degP[:, p])
    gb = node_core * NBLK + blk
    order_m = np.argsort(gb, kind="stable")
    bounds = np.searchsorted(gb[order_m], np.arange(NCORES * NBLK + 1))
    members = {cb: list(order_m[bounds[cb]:bounds[cb + 1]])
               for cb in range(NCORES * NBLK)}
    target = 8 * P
    for _ in range(3000):
        worst = int(np.argmax(L.max(axis=1)))
        if L[worst].max() <= target:
            break
        p_hot = int(np.argmax(L[worst]))
        c = worst // NBLK
        grp = (worst % NBLK) // GBLK
        best = None
        mem_w = sorted(members[worst], key=lambda n: -degP[n, p_hot])[:12]
        for j in range(GBLK):
            b2 = c * NBLK + grp * GBLK + j
            if b2 == worst:
                continue
            mem2 = sorted(members[b2], key=lambda n: degP[n, p_hot])[:12]
            for n in mem_w:
                for m in mem2:
                    nm = max(L[worst][0] - degP[n, 0] + degP[m, 0],
                             L[worst][1] - degP[n, 1] + degP[m, 1],
                             L[b2][0] + degP[n, 0] - degP[m, 0],
                             L[b2][1] + degP[n, 1] - degP[m, 1])
                    if best is None or nm < best[0]:
                        best = (nm, n, m, b2)
        if best is None or best[0] >= L[worst].max():
            break
        _, n, m, b2 = best
        members[worst].remove(n); members[b2].remove(m)
        members[worst].append(m); members[b2].append(n)
        for p in (0, 1):
            L[worst][p] += degP[m, p] - degP[n, p]
            L[b2][p] += degP[n, p] - degP[m, p]
        blk[n], blk[m] = blk[m], blk[n]
        slot[n], slot[m] = slot[m], slot[n]

    # row of each node inside its part's AllGather table
    agrow_p = node_core * PNPC + (blk - part * GBLK) * NPB + slot

    # group edges by (dst core, dst block, src part)
    ecore = (dst // NPC).astype(np.int32)
    eblk = blk[dst]
    edstl = slot[dst]
    epart = part[src].astype(np.int32)
    key = (ecore * NBLK + eblk) * 2 + epart
    order = np.lexsort((src, key))
    s_key = key[order]
    s_src = src[order]
    s_dstl = edstl[order].astype(np.float32)

    counts = np.bincount(key, minlength=NCORES * NBLK * 2)
    NCH = int(np.ceil(counts.max() / P))          # chunks per (block, part)
    SC = NBLK * NCH                               # stream chunks per part
    SCP = ((SC + CPG - 1) // CPG) * CPG           # padded to gather multiple
    NG = SCP // CPG                               # gathers per part-stream

    starts = np.concatenate([[0], np.cumsum(counts)])[:-1]
    rank = np.arange(len(s_key)) - starts[s_key]
    chunkrel = rank // P
    spart = s_key % 2
    score_blk = (s_key // 2) % NBLK               # dst block
    score_core = s_key // (2 * NBLK)              # dst core
    scol = score_blk * NCH + chunkrel             # stream chunk column
    flat = scol * P + (rank % P)                  # position within stream

    hrow = agrow_p[s_src].astype(np.int16)
    valid_f = np.ones(len(s_src), np.float32)

    in_maps = []
    xp = np.asarray(x, dtype=np.float32) * dinv[:, None]
    W1b = np.asarray(W1, dtype=np.float32).reshape(2, P, F).astype(np.float16)
    W2b = np.asarray(W2, dtype=np.float32).reshape(2, P, F).astype(np.float16)
    gb1 = np.concatenate([np.asarray(g1, np.float32),
                          np.asarray(be1, np.float32)])[None, :]
    gb2 = np.concatenate([np.asarray(g2, np.float32),
                          np.asarray(be2, np.float32)])[None, :]
    batch = np.asarray(batch, dtype=np.int64)

    for c in range(NCORES):
        m = {}
        for p in (0, 1):
            sel = (score_core == c) & (spart == p)
            vidx = np.zeros(SCP * P, np.int16)
            vdstl = np.zeros(SCP * P, np.float32)
            vvalid = np.zeros(SCP * P, np.float32)
            f = flat[sel]
            vidx[f] = hrow[sel]
            vdstl[f] = s_dstl[sel]
            vvalid[f] = valid_f[sel]
            # wrap idxs: idx i -> [i%16, i//16], replicated to 128 partitions
            w16 = vidx.reshape(-1, 16).T               # [16, SCP*8]
            m[f"idx{p}"] = np.ascontiguousarray(np.tile(w16, (8, 1)))
            # fp8 one-hot blob: Bb[q, scol*128 + d] = (dstl==d) & valid
            dstl2 = vdstl.reshape(SCP, P).T            # [128, SCP]
            valid = (vvalid.reshape(SCP, P).T != 0.0)
            oneh = (dstl2[:, :, None] ==
                    np.arange(P, dtype=np.float32)[None, None, :]) & valid[:, :, None]
            m[f"bb{p}"] = np.ascontiguousarray(
                oneh.reshape(P, SCP * P)).astype(ml_dtypes.float8_e4m3)

        nodes = np.arange(c * NPC, (c + 1) * NPC)
        col = blk[nodes] * P + slot[nodes]
        ddst = np.zeros((NBLK * P,), np.float32)
        ddst[col] = dinv[nodes]
        m["ddst"] = np.ascontiguousarray(ddst.reshape(NBLK, P).T)   # [128, NBLK]
        bt = np.full((NBLK * P,), 1000.0, np.float32)
        bt[col] = batch[nodes].astype(np.float32)
        m["bt"] = np.ascontiguousarray(bt.reshape(NBLK, P).T)       # [128, NBLK]

        xa = np.zeros((NBLK * P, F), np.float32)
        xa[col] = xp[nodes]
        m["xT"] = np.ascontiguousarray(
            xa.T.reshape(2, P, NBLK * P)).astype(np.float16)

        m["w1"] = W1b
        m["w2"] = W2b
        m["gb1"] = gb1
        m["gb2"] = gb2
        in_maps.append(m)

    cnt = np.bincount(batch, minlength=G).astype(np.float32)
    return in_maps, cnt, NCH, SCP, NG


# ----------------------------------------------------------------------------
# device program
# ----------------------------------------------------------------------------

def _build(cfg, NCH, SCP, NG, reps=1):
    N, G, NBLK, NPB, NPC = cfg.N, cfg.G, cfg.NBLK, cfg.NPB, cfg.NPC
    GBLK, PHALF = cfg.GBLK, cfg.PHALF
    rg = [list(range(NCORES))]

    nc = bacc.Bacc("TRN2", target_bir_lowering=False, debug=False,
                   num_devices=NCORES, num_swdge_queues=4)

    F8 = mybir.dt.float8e4
    din = {}
    for p in (0, 1):
        din[f"idx{p}"] = nc.dram_tensor(f"idx{p}", [P, SCP * 8], I16,
                                        kind="ExternalInput")
        din[f"bb{p}"] = nc.dram_tensor(f"bb{p}", [P, SCP * P], F8,
                                       kind="ExternalInput")
    din["ddst"] = nc.dram_tensor("ddst", [P, NBLK], F32, kind="ExternalInput")
    din["bt"] = nc.dram_tensor("bt", [P, NBLK], F32, kind="ExternalInput")
    din["xT"] = nc.dram_tensor("xT", [2, P, NBLK * P], F16, kind="ExternalInput")
    din["w1"] = nc.dram_tensor("w1", [2, P, F], F16, kind="ExternalInput")
    din["w2"] = nc.dram_tensor("w2", [2, P, F], F16, kind="ExternalInput")
    din["gb1"] = nc.dram_tensor("gb1", [1, 2 * F], F32, kind="ExternalInput")
    din["gb2"] = nc.dram_tensor("gb2", [1, 2 * F], F32, kind="ExternalInput")

    pool_out = nc.dram_tensor("pool_out", [G, F], F32, kind="ExternalOutput")

    ag_in = [[nc.dram_tensor(f"ag_in{l}_{p}", [NPC // 2, F], F16,
                             kind="Internal") for p in (0, 1)] for l in (0, 1)]
    ag_out = [[nc.dram_tensor(f"ag_out{l}_{p}", [PHALF, F], F16,
                              kind="Internal", addr_space="Shared")
               for p in (0, 1)] for l in (0, 1)]
    st_in = [nc.dram_tensor(f"st_in{l}", [1, 2 * F], F32, kind="Internal")
             for l in (0, 1)]
    st_out = [nc.dram_tensor(f"st_out{l}", [1, 2 * F], F32, kind="Internal",
                             addr_space="Shared") for l in (0, 1)]

    with tile.TileContext(nc) as tc:
        import contextlib
        with contextlib.ExitStack() as ctx:
            meta = ctx.enter_context(tc.tile_pool(name="meta", bufs=1))
            big = ctx.enter_context(tc.tile_pool(name="big", bufs=1))
            gpools = [ctx.enter_context(tc.tile_pool(name=f"g{p}", bufs=9))
                      for p in (0, 1)]
            bpool = ctx.enter_context(tc.tile_pool(name="bpool", bufs=8))
            wpool = ctx.enter_context(tc.tile_pool(name="wpool", bufs=3))
            spool = ctx.enter_context(tc.tile_pool(name="spool", bufs=2))
            ps_agg = ctx.enter_context(
                tc.tile_pool(name="ps_agg", bufs=3, space="PSUM"))
            ps_st = ctx.enter_context(
                tc.tile_pool(name="ps_st", bufs=1, space="PSUM"))
            ps_misc = ctx.enter_context(
                tc.tile_pool(name="ps_misc", bufs=2, space="PSUM"))
            ps_pool = ctx.enter_context(
                tc.tile_pool(name="ps_pool", bufs=1, space="PSUM"))

            # --- resident data
            hT1 = [big.tile([P, NBLK * P], F16, tag=f"hT1_{kc}", name=f"hT1_{kc}")
                   for kc in (0, 1)]
            for kc in (0, 1):
                nc.sync.dma_start(out=hT1[kc][:], in_=din["xT"][kc, :, :])
            w_t = []
            for l, name in ((0, "w1"), (1, "w2")):
                tiles = []
                for kc in (0, 1):
                    wt = meta.tile([P, F], F16, tag=f"{name}_{kc}", name=f"{name}_{kc}")
                    nc.sync.dma_start(out=wt[:], in_=din[name][kc, :, :])
                    tiles.append(wt)
                w_t.append(tiles)
            idx_t = []
            for p in (0, 1):
                it = meta.tile([P, SCP * 8], I16, tag=f"idx{p}", name=f"idx{p}")
                nc.sync.dma_start(out=it[:], in_=din[f"idx{p}"][:, :])
                idx_t.append(it)
            ddst_t = meta.tile([P, NBLK], F32, tag="ddst", name="ddst")
            nc.sync.dma_start(out=ddst_t[:], in_=din["ddst"][:, :])
            bt_t = meta.tile([P, NBLK], F32, tag="bt", name="bt")
            nc.sync.dma_start(out=bt_t[:], in_=din["bt"][:, :])

            gb_t = []
            for l, name in ((0, "gb1"), (1, "gb2")):
                gt = meta.tile([1, 2 * F], F32, tag=name, name=name)
                nc.sync.dma_start(out=gt[:], in_=din[name][:, :])
                gb_t.append(gt)

            iota_i = meta.tile([P, P], I32, tag="iota_i", name="iota_i")
            nc.gpsimd.iota(iota_i[:], [[1, P]], channel_multiplier=0)
            iota_f = meta.tile([P, P], F32, tag="iota_f", name="iota_f")
            nc.vector.tensor_copy(out=iota_f[:], in_=iota_i[:])

            from concourse.masks import make_identity
            ident = meta.tile([P, P], F16, tag="ident", name="ident")
            make_identity(nc, ident[:])

            vmask = meta.tile([P, 1], F16, tag="vmask", name="vmask")
            nc.vector.memset(vmask[:], 0.0)
            nc.vector.memset(vmask[0:NPB, :], 1.0)

            eps_t = meta.tile([1, 1], F32, tag="eps_t", name="eps_t")
            nc.vector.memset(eps_t[:], BN_EPS)

            hT2 = [big.tile([P, NBLK * P], F16, tag=f"hT2_{kc}", name=f"hT2_{kc}")
                   for kc in (0, 1)]
            z_all = big.tile([P, NBLK * F], F16, tag="z_all", name="z_all")

            recip_n = 1.0 / float(N)

            def fire_ag(layer, p):
                if DEBUG_NO_CC:
                    nc.sync.dma_start(
                        out=ag_out[layer][p][0:NPC // 2, :],
                        in_=ag_in[layer][p][:, :])
                else:
                    nc.gpsimd.collective_compute(
                        "AllGather", mybir.AluOpType.bypass, replica_groups=rg,
                        ins=[ag_in[layer][p][:, :]],
                        outs=[ag_out[layer][p][:, :]])

            for rep in range(reps):
                # --- layer-0 table: per-group matmuls + early AllGathers ----
                for p in (0, 1):
                    for b in range(p * GBLK, (p + 1) * GBLK):
                        mp = ps_misc.tile([P, F], F32, tag="misc", name="misc")
                        for kc in (0, 1):
                            nc.tensor.matmul(
                                out=mp[:], lhsT=hT1[kc][:, b * P:(b + 1) * P],
                                rhs=w_t[0][kc][:],
                                start=(kc == 0), stop=(kc == 1))
                        m_sb = wpool.tile([P, F], F16, tag="m_sb", name="m_sb")
                        nc.vector.tensor_copy(out=m_sb[:], in_=mp[:])
                        nc.sync.dma_start(
                            out=ag_in[0][p][(b - p * GBLK) * NPB:
                                            (b - p * GBLK + 1) * NPB, :],
                            in_=m_sb[0:NPB, :])
                    fire_ag(0, p)

                for layer in (0, 1):
                    # --- aggregation: one sweep per source part -------------
                    gtiles = {0: {}, 1: {}}
                    btiles = {0: {}, 1: {}}

                    def ensure_gather(p, gi, layer=layer, gtiles=gtiles):
                        if gi in gtiles[p]:
                            return gtiles[p][gi]
                        gt = gpools[p].tile([P, CPG * F], F16, tag=f"gt{p}",
                                            name=f"gt{p}")
                        nc.gpsimd.dma_gather(
                            out_ap=gt[:].rearrange("p (c d) -> p c d", d=F),
                            in_ap=ag_out[layer][p][:, :],
                            idxs_ap=idx_t[p][:, gi * (NI // 16):(gi + 1) * (NI // 16)],
                            num_idxs=NI, num_idxs_reg=NI, elem_size=F,
                            queue_num=gi % 4)
                        gtiles[p][gi] = gt
                        return gt

                    def ensure_btile(p, gi, btiles=btiles):
                        if gi in btiles[p]:
                            return btiles[p][gi]
                        bt_ = bpool.tile([P, CPG * P], F8, tag=f"bb{p}",
                                         name=f"bb{p}")
                        nc.sync.dma_start(
                            out=bt_[:],
                            in_=din[f"bb{p}"][:, gi * CPG * P:(gi + 1) * CPG * P])
                        btiles[p][gi] = bt_
                        return bt_

                    hTl = hT1 if layer == 0 else hT2
                    for p in (0, 1):
                        for b in range(NBLK):
                            agg = ps_agg.tile([P, F], F32, tag="agg", name="agg")
                            if p == 0:
                                # self-loop term: this block's own table rows,
                                # recomputed from the resident transposed input
                                for kc in (0, 1):
                                    nc.tensor.matmul(
                                        out=agg[:],
                                        lhsT=hTl[kc][:, b * P:(b + 1) * P],
                                        rhs=w_t[layer][kc][:],
                                        start=(kc == 0), stop=False)
                            for j in range(NCH):
                                scol = b * NCH + j
                                gi, gslot = divmod(scol, CPG)
                                gt = ensure_gather(p, gi)
                                bt_ = ensure_btile(p, gi)
                                nc.tensor.matmul(
                                    out=agg[:],
                                    lhsT=bt_[:, gslot * P:(gslot + 1) * P],
                                    rhs=gt[:, gslot * F:(gslot + 1) * F],
                                    start=(p == 0 and False) or (p == 1 and j == 0),
                                    stop=(j == NCH - 1))
                            zsl = z_all[:, b * F:(b + 1) * F]
                            if p == 0:
                                nc.scalar.activation(
                                    out=zsl, in_=agg[:],
                                    func=mybir.ActivationFunctionType.Copy,
                                    scale=ddst_t[:, b:b + 1])
                            else:
                                zt = wpool.tile([P, F], F16, tag="zt",
                                                name="zt", bufs=4)
                                nc.scalar.activation(
                                    out=zt[:], in_=agg[:],
                                    func=mybir.ActivationFunctionType.Copy,
                                    scale=ddst_t[:, b:b + 1])
                                nc.vector.tensor_tensor(
                                    out=zsl, in0=zt[:], in1=zsl,
                                    op=mybir.AluOpType.add)

                    # --- BN stats: end-of-layer sweep ------------------------
                    ssum = ps_st.tile([1, F], F32, tag="ssum", name="ssum")
                    ssq = ps_st.tile([1, F], F32, tag="ssq", name="ssq")
                    for b in range(NBLK):
                        zsl = z_all[:, b * F:(b + 1) * F]
                        sq_t = wpool.tile([P, F], F16, tag="sq_t", name="sq_t",
                                          bufs=4)
                        nc.vector.tensor_tensor(out=sq_t[:], in0=zsl, in1=zsl,
                                                op=mybir.AluOpType.mult)
                        nc.tensor.matmul(out=ssum[:], lhsT=vmask[:], rhs=zsl,
                                         start=(b == 0), stop=(b == NBLK - 1))
                        nc.tensor.matmul(out=ssq[:], lhsT=vmask[:], rhs=sq_t[:],
                                         start=(b == 0), stop=(b == NBLK - 1))

                    # --- stats AllReduce + scale/shift ------------------------
                    srow = spool.tile([1, 2 * F], F32, tag="srow", name="srow")
                    nc.vector.tensor_copy(out=srow[:, 0:F], in_=ssum[:])
                    nc.vector.tensor_copy(out=srow[:, F:2 * F], in_=ssq[:])
                    nc.sync.dma_start(out=st_in[layer][:, :], in_=srow[:])
                    if DEBUG_NO_CC:
                        nc.sync.dma_start(out=st_out[layer][:, :],
                                          in_=st_in[layer][:, :])
                    else:
                        nc.gpsimd.collective_compute(
                            "AllReduce", mybir.AluOpType.add, replica_groups=rg,
                            ins=[st_in[layer][:, :]], outs=[st_out[layer][:, :]])
                    srow2 = spool.tile([1, 2 * F], F32, tag="srow2", name="srow2")
                    nc.sync.dma_start(out=srow2[:], in_=st_out[layer][:, :])

                    mu = spool.tile([1, F], F32, tag="mu", name="mu")
                    nc.vector.tensor_scalar(out=mu[:], in0=srow2[:, 0:F],
                                            scalar1=recip_n, scalar2=None,
                                            op0=mybir.AluOpType.mult)
                    ex2 = spool.tile([1, F], F32, tag="ex2", name="ex2")
                    nc.vector.tensor_scalar(out=ex2[:], in0=srow2[:, F:2 * F],
                                            scalar1=recip_n, scalar2=None,
                                            op0=mybir.AluOpType.mult)
                    var = spool.tile([1, F], F32, tag="var", name="var")
                    nc.vector.tensor_tensor(out=var[:], in0=mu[:], in1=mu[:],
                                            op=mybir.AluOpType.mult)
                    nc.vector.tensor_tensor(out=var[:], in0=ex2[:], in1=var[:],
                                            op=mybir.AluOpType.subtract)
                    sd = spool.tile([1, F], F32, tag="sd", name="sd")
                    nc.scalar.activation(out=sd[:], in_=var[:],
                                         func=mybir.ActivationFunctionType.Sqrt,
                                         bias=eps_t[:])
                    rstd = spool.tile([1, F], F32, tag="rstd", name="rstd")
                    nc.vector.reciprocal(rstd[:], sd[:])
                    s_row = spool.tile([1, F], F32, tag="s_row", name="s_row")
                    nc.vector.tensor_tensor(out=s_row[:], in0=rstd[:],
                                            in1=gb_t[layer][:, 0:F],
                                            op=mybir.AluOpType.mult)
                    t_row = spool.tile([1, F], F32, tag="t_row", name="t_row")
                    nc.vector.tensor_tensor(out=t_row[:], in0=mu[:], in1=s_row[:],
                                            op=mybir.AluOpType.mult)
                    nc.vector.tensor_tensor(out=t_row[:], in0=gb_t[layer][:, F:2 * F],
                                            in1=t_row[:],
                                            op=mybir.AluOpType.subtract)
                    S_b = spool.tile([P, F], F32, tag="S_b", name="S_b")
                    nc.gpsimd.partition_broadcast(out_ap=S_b[:], in_ap=s_row[:])
                    T_b = spool.tile([P, F], F32, tag="T_b", name="T_b")
                    nc.gpsimd.partition_broadcast(out_ap=T_b[:], in_ap=t_row[:])

                    # --- pass 2: h = relu(z*S + T), feed next stage ----------
                    if layer == 1:
                        pool_ps = ps_pool.tile([G, F], F32, tag="pool", name="pool")
                    for p in (0, 1):
                        for b in range(p * GBLK, (p + 1) * GBLK):
                            zsl = z_all[:, b * F:(b + 1) * F]
                            u = wpool.tile([P, F], F16, tag="u", name="u", bufs=4)
                            nc.vector.tensor_tensor(out=u[:], in0=zsl, in1=S_b[:],
                                                    op=mybir.AluOpType.mult)
                            u2 = wpool.tile([P, F], F16, tag="u2", name="u2", bufs=4)
                            nc.vector.tensor_tensor(out=u2[:], in0=u[:], in1=T_b[:],
                                                    op=mybir.AluOpType.add)
                            hp = wpool.tile([P, F], F16, tag="hp", name="hp")
                            if layer == 0:
                                nc.scalar.activation(
                                    out=hp[:], in_=u2[:],
                                    func=mybir.ActivationFunctionType.Relu,
                                    scale=ddst_t[:, b:b + 1])
                                for kc in (0, 1):
                                    tp = ps_misc.tile([P, P], F16, tag="misc", name="misc")
                                    nc.tensor.transpose(
                                        out=tp[:], in_=hp[:, kc * P:(kc + 1) * P],
                                        identity=ident[:])
                                    nc.vector.tensor_copy(
                                        out=hT2[kc][:, b * P:(b + 1) * P], in_=tp[:])
                                mp2 = ps_misc.tile([P, F], F32, tag="misc", name="misc")
                                for kc in (0, 1):
                                    nc.tensor.matmul(
                                        out=mp2[:], lhsT=hT2[kc][:, b * P:(b + 1) * P],
                                        rhs=w_t[1][kc][:],
                                        start=(kc == 0), stop=(kc == 1))
                                m_sb2 = wpool.tile([P, F], F16, tag="m_sb", name="m_sb")
                                nc.vector.tensor_copy(out=m_sb2[:], in_=mp2[:])
                                nc.sync.dma_start(
                                    out=ag_in[1][p][(b - p * GBLK) * NPB:
                                                    (b - p * GBLK + 1) * NPB, :],
                                    in_=m_sb2[0:NPB, :])
                            else:
                                nc.scalar.activation(
                                    out=hp[:], in_=u2[:],
                                    func=mybir.ActivationFunctionType.Relu)
                                pone = bpool.tile([P, G], F16, tag="pone", name="pone")
                                nc.vector.tensor_scalar(
                                    out=pone[:], in0=iota_f[:, 0:G],
                                    scalar1=bt_t[:, b:b + 1], scalar2=None,
                                    op0=mybir.AluOpType.is_equal)
                                nc.tensor.matmul(out=pool_ps[:], lhsT=pone[:],
                                                 rhs=hp[:], start=(b == 0),
                                                 stop=(b == NBLK - 1))
                        if layer == 0:
                            fire_ag(1, p)

                pool_sb = spool.tile([G, F], F32, tag="pool_sb", name="pool_sb")
                nc.vector.tensor_copy(out=pool_sb[:], in_=pool_ps[:])
                nc.sync.dma_start(out=pool_out[:, :], in_=pool_sb[:])

    nc.compile()
    return nc


DEBUG_NO_CC = False   # replace collectives with local DMA (timing-only builds)

_CACHE = {}


def _get_program(cfg, NCH, SCP, NG, reps=1):
    key = (cfg.N, cfg.G, cfg.NBLK, cfg.NPB, NCH, SCP, NG, reps)
    if key not in _CACHE:
        _CACHE[key] = _build(cfg, NCH, SCP, NG, reps)
    return _CACHE[key]


def _run(inputs, cfg, trace=False):
    in_maps, cnt, NCH, SCP, NG = _preprocess(
        inputs["x"], inputs["ei"], inputs["batch"],
        inputs["W1"], inputs["g1"], inputs["be1"],
        inputs["W2"], inputs["g2"], inputs["be2"], cfg)
    nc = _get_program(cfg, NCH, SCP, NG)
    res = bass_utils.run_bass_kernel_spmd(
        nc, in_maps, core_ids=list(range(NCORES)), trace=trace)
    partial = np.zeros((cfg.G, F), np.float32)
    for c in range(NCORES):
        partial += np.asarray(res.results[c]["pool_out"], np.float32)
    out = partial / np.maximum(cnt, 1.0)[:, None]
    return out.astype(np.float32), res


def kernel(**inputs):
    cfg = Cfg(N=40000, G=64, NBLK=40, NPB=125)
    out, _ = _run(inputs, cfg)
    return out
